# revision 1
# baseline (speedup 1.0000x reference)
"""Bahdanau attention fused kernel for Trainium2, 8-core data-parallel.

Reference computation (per batch b of 32, H=1024, S=2048):
    enc_score = encoder_out @ We + be                    [B, S, H]
    dec_score = dec @ Wd + bd                            [B, 1, H]
    score     = tanh(enc_score + dec_score)              [B, S, H]
    ls        = score @ Ws + bs                          [B, S, 1]
    w         = softmax(ls, axis=S)
    out       = sum_s w[b,s] * encoder_out[b,s,:]        [B, H]

Sharding: batch 32 -> 4 per core across 8 cores; weights replicated.
The tiny dec-score GEMM (67 MFLOP of 137 GFLOP, 0.05%) is folded into the
host-side bias preparation: bias[b] = be + bd + dec[b] @ Wd. bs is dropped
(softmax is shift-invariant). No max-subtraction in softmax: |ls| <= 16.

Per-core device layout (everything h-partitioned, prepared host-side):
    xt   [4, 4, 128, 8*512] bf16  xt[b, c, p, k*512+s'] = X[b, c*512+s', k*128+p]
    we   [128, 8*1024]      bf16  we[p, k*1024+n]       = We[k*128+p, n]
    ws   [128, 8]           bf16  ws[p, j]              = Ws[j*128+p, 0]
    bias [128, 32]          f32   bias[p, j*4+b]        = (be+bd+dec[b]@Wd)[j*128+p]
    out: ctx [4, 128, 8]    f32   ctx[b, p, j]          = out[b, j*128+p]

Device schedule per batch b (PE-bound, ~221us roofline/core at bf16):
  - enc_score.T tiles via matmul: We (stationary) x X.T (moving), 8 k-tiles
    accumulated in PSUM -> [128 h_out, 512 s]; issue cadence is the N=512
    streaming limit (~216 ns/matmul)
  - ScalarE evacuates PSUM with fused tanh(psum + bias[b,j]) -> bf16
  - ls.T = sum_j Ws[j-tile].T @ tanh-tile, accumulated in PSUM [1, 512]
  - ScalarE exp (bf16) with fused accum_out denominator (fp32)
  - ONLINE context: per s-chunk, raw exp weights are broadcast to 128
    partitions via a ones-matmul, multiplied against the cached X.T chunk
    (VectorE) and partial-reduced per k-tile (VectorE; ScalarE accum_out
    for each batch's last chunk); the softmax denominator is divided out
    once per batch. The context of chunk c is emitted after the matmuls of
    chunk c+1 so the PE never waits on the softmax chain.
"""

import numpy as np
import ml_dtypes

import concourse.tile as tile
from concourse import bacc, mybir
from concourse.bass_utils import run_bass_kernel_spmd

BF16 = mybir.dt.bfloat16
F32 = mybir.dt.float32
AF = mybir.ActivationFunctionType

N_CORES = 8
H = 1024
S = 2048
B_PER_CORE = 4
S_CHUNK = 512

# test.py can flip this to get a profiled run; the grading path never does.
PROFILE = {"trace": False, "tmpdir": None}


def build_program(b_per_core=B_PER_CORE, s=S, h=H):
    kt = h // 128
    jt = h // 128
    n_sc = s // S_CHUNK
    nc = bacc.Bacc("TRN2", target_bir_lowering=False, debug=False)

    xt_d = nc.dram_tensor(
        "xt", [b_per_core, n_sc, 128, kt * S_CHUNK], BF16, kind="ExternalInput"
    ).ap()
    we_d = nc.dram_tensor("we", [128, kt * h], BF16, kind="ExternalInput").ap()
    ws_d = nc.dram_tensor("ws", [128, jt], BF16, kind="ExternalInput").ap()
    bias_d = nc.dram_tensor(
        "bias", [128, jt * b_per_core], F32, kind="ExternalInput"
    ).ap()
    ctx_d = nc.dram_tensor("ctx", [b_per_core, 128, jt], F32, kind="ExternalOutput").ap()

    with tile.TileContext(nc) as tc:
        with (
            tc.tile_pool(name="consts", bufs=1) as consts,
            tc.tile_pool(name="xtp", bufs=12) as xtp,
            tc.tile_pool(name="scorep", bufs=10) as scorep,
            tc.tile_pool(name="smallp", bufs=2 * n_sc) as smallp,
            tc.tile_pool(name="ebcp", bufs=2 * n_sc) as ebcp,
            tc.tile_pool(name="scrp", bufs=6) as scrp,
            tc.tile_pool(name="trashp", bufs=1) as trashp,
            tc.tile_pool(name="ctxp", bufs=2) as ctxp,
            tc.tile_pool(name="ps_main", bufs=4, space="PSUM") as ps_main,
            tc.tile_pool(name="ps_ls", bufs=2, space="PSUM") as ps_ls,
            tc.tile_pool(name="ps_misc", bufs=2, space="PSUM") as ps_misc,
        ):
            # we goes FIRST on the sync ring, ahead of the xt stream: with
            # the scalar ring nearly empty, the sync ring gets all 16 SDMA
            # engines, so the first-matmul gate (we + xt[0,0]) clears at
            # full HBM bandwidth instead of splitting it with prefetch.
            we_sb = consts.tile([128, kt * h], BF16)
            nc.sync.dma_start(we_sb[:], we_d[:])
            ws_sb = consts.tile([128, jt], BF16)
            nc.scalar.dma_start(ws_sb[:], ws_d[:])
            bias_sb = consts.tile([128, jt * b_per_core], F32)
            nc.scalar.dma_start(bias_sb[:], bias_d[:])
            ones_bf = consts.tile([1, 128], BF16)
            nc.vector.memset(ones_bf[:], 1.0)
            ones_f32 = consts.tile([1, 128], F32)
            nc.vector.memset(ones_f32[:], 1.0)

            def emit_context_chunk(xt_bc, ex, ctx4_b, c, last_chunk, tail=False):
                """Broadcast chunk weights and accumulate context partials.

                The broadcast runs on the otherwise-idle GpSimd engine except
                on the kernel's final chunk, where the PE is idle and the
                ones-matmul + cast path has lower latency.
                """
                ebc = ebcp.tile([128, S_CHUNK], BF16, tag="ebc")
                if tail:
                    bc_ps = ps_misc.tile([128, S_CHUNK], F32, tag="misc")
                    nc.tensor.matmul(
                        bc_ps[:], lhsT=ones_bf[:], rhs=ex[:], start=True, stop=True
                    )
                    nc.vector.tensor_copy(ebc[:], bc_ps[:])
                else:
                    nc.gpsimd.partition_broadcast(ebc[:], ex[:])
                for k in range(kt):
                    scr = scrp.tile([128, S_CHUNK], BF16, tag="scr")
                    nc.vector.tensor_mul(
                        scr[:], xt_bc[:, k * S_CHUNK : (k + 1) * S_CHUNK], ebc[:]
                    )
                    if last_chunk and k % 2 == 0:
                        trash = trashp.tile([128, S_CHUNK], BF16, tag="trash")
                        nc.scalar.activation(
                            trash[:], scr[:], AF.Identity,
                            accum_out=ctx4_b[:, k * n_sc + c : k * n_sc + c + 1],
                        )
                    else:
                        nc.vector.reduce_sum(
                            ctx4_b[:, k * n_sc + c : k * n_sc + c + 1],
                            scr[:],
                            axis=mybir.AxisListType.X,
                        )

            def emit_invd(denom_b):
                """softmax denominator -> broadcast 1/d [128, 1]."""
                dsum = smallp.tile([1, 1], F32, tag="dsum")
                nc.vector.reduce_sum(dsum[:], denom_b[:], axis=mybir.AxisListType.X)
                invd = smallp.tile([1, 1], F32, tag="invd")
                nc.vector.reciprocal(invd[:], dsum[:])
                iv_ps = ps_misc.tile([128, S_CHUNK], F32, tag="misc")
                nc.tensor.matmul(
                    iv_ps[:, 0:1], lhsT=ones_f32[:], rhs=invd[:], start=True, stop=True
                )
                invd_bc = smallp.tile([128, 1], F32, tag="invdbc")
                nc.scalar.copy(invd_bc[:], iv_ps[:, 0:1])
                return invd_bc

            def emit_batch_final(b, ctx4_b, invd_bc):
                """Partial reduction, normalize, store."""
                ctxu = ctxp.tile([128, jt], F32, tag="ctxu")
                for k in range(kt):
                    nc.vector.reduce_sum(
                        ctxu[:, k : k + 1],
                        ctx4_b[:, k * n_sc : (k + 1) * n_sc],
                        axis=mybir.AxisListType.X,
                    )
                ctx_b = ctxp.tile([128, jt], F32, tag="ctx")
                nc.vector.tensor_scalar_mul(ctx_b[:], ctxu[:], invd_bc[:])
                nc.sync.dma_start(ctx_d[b], ctx_b[:])

            pending = []  # deferred (context-chunk | invd | batch-final)
            for b in range(b_per_core):
                xt_tiles = []
                for c in range(n_sc):
                    xt_bc = xtp.tile([128, kt * S_CHUNK], BF16, tag="xt")
                    if b == 0 and c == 0:
                        # split the gate-opening chunk so the first matmul
                        # group starts on the early half
                        half = kt // 2 * S_CHUNK
                        nc.sync.dma_start(xt_bc[:, :half], xt_d[b, c][:, :half])
                        nc.sync.dma_start(xt_bc[:, half:], xt_d[b, c][:, half:])
                    else:
                        nc.sync.dma_start(xt_bc[:], xt_d[b, c])
                    xt_tiles.append(xt_bc)

                denom_b = smallp.tile([1, n_sc], F32, tag="denom")
                ctx4_b = ctxp.tile([128, kt * n_sc], F32, tag="ctx4")
                for c in range(n_sc):
                    ls_ps = ps_ls.tile([1, S_CHUNK], F32, tag="ls")
                    score_tiles = []
                    for j in range(jt):
                        mm_ps = ps_main.tile([128, S_CHUNK], F32, tag="main")
                        for k in range(kt):
                            nc.tensor.matmul(
                                mm_ps[:],
                                lhsT=we_sb[:, k * h + j * 128 : k * h + (j + 1) * 128],
                                rhs=xt_tiles[c][:, k * S_CHUNK : (k + 1) * S_CHUNK],
                                start=(k == 0),
                                stop=(k == kt - 1),
                            )
                        sc = scorep.tile([128, S_CHUNK], BF16, tag="score")
                        nc.scalar.activation(
                            sc[:], mm_ps[:], AF.Tanh,
                            bias=bias_sb[:, j * b_per_core + b : j * b_per_core + b + 1],
                        )
                        score_tiles.append(sc)
                        if j == 0:
                            # deferred work from the previous chunk/batch is
                            # emitted right after the first matmul group, so
                            # its PE ops (weight broadcast) slot in early and
                            # the DVE context work overlaps this chunk's
                            # remaining matmul groups
                            for fn in pending:
                                fn()
                            pending = []
                    for j in range(jt):
                        nc.tensor.matmul(
                            ls_ps[:],
                            lhsT=ws_sb[:, j : j + 1],
                            rhs=score_tiles[j][:],
                            start=(j == 0),
                            stop=(j == jt - 1),
                        )
                    ex = smallp.tile([1, S_CHUNK], BF16, tag="exp")
                    nc.scalar.activation(
                        ex[:], ls_ps[:], AF.Exp, accum_out=denom_b[:, c : c + 1]
                    )

                    last_b = b == b_per_core - 1
                    ctx_fn = (
                        lambda xt_bc=xt_tiles[c], ex=ex, ctx4_b=ctx4_b, c=c,
                        lc=(c == n_sc - 1), tl=(last_b and c == n_sc - 1):
                        emit_context_chunk(xt_bc, ex, ctx4_b, c, lc, tail=tl)
                    )
                    if c < n_sc - 1:
                        pending.append(ctx_fn)
                    elif last_b:
                        # tail of the whole kernel: get 1/d going on the
                        # still-empty DVE queue, then the final context chunk
                        invd_bc = emit_invd(denom_b)
                        ctx_fn()
                        emit_batch_final(b, ctx4_b, invd_bc)
                    else:
                        def batch_tail(ctx_fn=ctx_fn, b=b, ctx4_b=ctx4_b,
                                       denom_b=denom_b):
                            invd_bc = emit_invd(denom_b)
                            ctx_fn()
                            emit_batch_final(b, ctx4_b, invd_bc)
                        pending.append(batch_tail)

    nc.compile()
    return nc


_CACHED = {}


def _get_program(key):
    if key not in _CACHED:
        _CACHED[key] = build_program(*key)
    return _CACHED[key]


def make_in_maps(encoder_out, decoder_hidden_state, We, be, Wd, bd, Ws, bs,
                 b_per_core=B_PER_CORE, s=S, h=H, n_cores=N_CORES):
    kt = h // 128
    jt = h // 128
    n_sc = s // S_CHUNK
    bf = ml_dtypes.bfloat16

    we_a = np.ascontiguousarray(
        We.reshape(kt, 128, h).transpose(1, 0, 2).reshape(128, kt * h)
    ).astype(bf)
    ws_a = np.ascontiguousarray(Ws[:, 0].reshape(jt, 128).T).astype(bf)

    dec = decoder_hidden_state[0]  # [32, h]
    bias_all = (be + bd)[None, :] + dec @ Wd  # [32, h] fp32
    in_maps = []
    for i in range(n_cores):
        b0 = i * b_per_core
        xb = encoder_out[b0 : b0 + b_per_core]  # [b, s, h]
        # [b, c, s', k, p] -> [b, c, p, k, s']
        xt_a = np.ascontiguousarray(
            xb.reshape(b_per_core, n_sc, S_CHUNK, kt, 128).transpose(0, 1, 4, 3, 2)
        ).reshape(b_per_core, n_sc, 128, kt * S_CHUNK).astype(bf)
        bias_a = np.ascontiguousarray(
            bias_all[b0 : b0 + b_per_core].reshape(b_per_core, jt, 128).transpose(2, 1, 0)
        ).reshape(128, jt * b_per_core).astype(np.float32)
        in_maps.append({"xt": xt_a, "we": we_a, "ws": ws_a, "bias": bias_a})
    return in_maps


def kernel(encoder_out, decoder_hidden_state, We, be, Wd, bd, Ws, bs):
    encoder_out = np.asarray(encoder_out, dtype=np.float32)
    decoder_hidden_state = np.asarray(decoder_hidden_state, dtype=np.float32)
    We = np.asarray(We, dtype=np.float32)
    be = np.asarray(be, dtype=np.float32)
    Wd = np.asarray(Wd, dtype=np.float32)
    bd = np.asarray(bd, dtype=np.float32)
    Ws = np.asarray(Ws, dtype=np.float32)
    bs = np.asarray(bs, dtype=np.float32)

    nc = _get_program((B_PER_CORE, S, H))
    in_maps = make_in_maps(
        encoder_out, decoder_hidden_state, We, be, Wd, bd, Ws, bs
    )
    kwargs = {}
    if PROFILE["trace"]:
        kwargs = {"trace": True, "tmpdir": PROFILE["tmpdir"]}
    res = run_bass_kernel_spmd(nc, in_maps, list(range(N_CORES)), **kwargs)
    PROFILE["last_result"] = res

    out = np.empty((N_CORES * B_PER_CORE, H), dtype=np.float32)
    for i in range(N_CORES):
        ctx = res.results[i]["ctx"]  # [b, 128, jt]
        out[i * B_PER_CORE : (i + 1) * B_PER_CORE] = (
            ctx.transpose(0, 2, 1).reshape(B_PER_CORE, H)
        )
    return out



# revision 2
# speedup vs baseline: 1.4083x; 1.4083x over previous
"""Bahdanau attention fused kernel for Trainium2, 8-core data-parallel.

Reference computation (per batch b of 32, H=1024, S=2048):
    enc_score = encoder_out @ We + be                    [B, S, H]
    dec_score = dec @ Wd + bd                            [B, 1, H]
    score     = tanh(enc_score + dec_score)              [B, S, H]
    ls        = score @ Ws + bs                          [B, S, 1]
    w         = softmax(ls, axis=S)
    out       = sum_s w[b,s] * encoder_out[b,s,:]        [B, H]

Sharding: batch 32 -> 4 per core across 8 cores; weights replicated.
The tiny dec-score GEMM is folded into the host-side bias preparation:
bias[b] = be + bd + dec[b] @ Wd. bs is dropped (softmax shift-invariant).

The main GEMM (enc_score) runs in fp8-e4m3 with MatmulPerfMode.DoubleRow:
the PE processes two 128-deep k-tiles per matmul (2 fp8 weights/cell), so
the 1024-deep contraction takes 4 matmuls instead of 8. We is pre-scaled
by 128 host-side to keep its +-1/32 values out of fp8 subnormals; the
tanh activation applies scale=1/128 to compensate. X is shipped twice:
fp8 (PE main GEMM) and bf16 (VectorE context path; fp8 would cost ~2.4%
relative error on the context).

Per-core device layout (prepared host-side):
    xt8  [4, 4, 128, 4096] fp8  xt8[b,c,p,t*1024+r*512+s'] = X[b, c*512+s', (2t+r)*128+p]
    xtb  [4, 4, 128, 4096] bf16 xtb[b,c,p,k*512+s']        = X[b, c*512+s', k*128+p]
    we8  [128, 8, 1024]    fp8  we8[p,j,t*256+r*128+m]     = 128*We[(2t+r)*128+p, j*128+m]
    ws   [128, 8]          bf16 ws[p, j]                   = Ws[j*128+p, 0]
    bias [128, 32]         f32  bias[p, j*4+b]             = (be+bd+dec[b]@Wd)[j*128+p]
    out: ctx [4, 128, 8]   f32  ctx[b, p, j]               = out[b, j*128+p]

Device schedule per (batch, 512-wide s-chunk):
  - 8 j-groups x 4 DoubleRow matmuls accumulate enc_score.T in PSUM;
    ScalarE evacuates with fused tanh(psum/128 + bias) -> bf16 scores
  - ls.T = sum_j Ws[j].T @ score_j in PSUM [1,512] (bf16 matmuls)
  - ScalarE exp with fused accum_out denominator
  - context: exp weights broadcast to 128 partitions (GpSimd; PE
    ones-matmul on the kernel tail), then ONE fused VectorE multiply
    [128,8,512] (broadcast-AP weights) + ONE fused reduce -> [128,8]
    per-k partials; deferred one chunk so the PE never waits
  - per batch: softmax denominator reciprocal, partial-sum over chunks,
    normalize, store

Startup: we8 is DMA'd in per-j slabs (j0,j1 ahead of the first xt8
chunk, which lands in 4 k-pair slabs) so the first matmul gates on
~0.4MB instead of 3MB.
"""

import numpy as np
import ml_dtypes

import concourse.tile as tile
from concourse import bacc, mybir
from concourse.bass_utils import run_bass_kernel_spmd

BF16 = mybir.dt.bfloat16
F32 = mybir.dt.float32
FP8 = mybir.dt.float8e4
AF = mybir.ActivationFunctionType
DR = mybir.MatmulPerfMode.DoubleRow

N_CORES = 8
H = 1024
S = 2048
B_PER_CORE = 4
S_CHUNK = 512
WE_SCALE = 128.0

# test.py can flip this to get a profiled run; the grading path never does.
PROFILE = {"trace": False, "tmpdir": None}


def build_program(b_per_core=B_PER_CORE, s=S, h=H):
    kt = h // 128          # 8  bf16 k-tiles (context path)
    kt2 = kt // 2          # 4  fp8 DoubleRow k-pair tiles
    jt = h // 128          # 8  output h tiles
    n_sc = s // S_CHUNK    # 4  s chunks
    nc = bacc.Bacc("TRN2", target_bir_lowering=False, debug=False)

    xt8_d = nc.dram_tensor(
        "xt8", [b_per_core, n_sc, 128, kt2 * 2 * S_CHUNK], FP8, kind="ExternalInput"
    ).ap()
    xtb_d = nc.dram_tensor(
        "xtb", [b_per_core, n_sc, 128, kt * S_CHUNK], BF16, kind="ExternalInput"
    ).ap()
    we8_d = nc.dram_tensor("we8", [128, jt, h], FP8, kind="ExternalInput").ap()
    ws_d = nc.dram_tensor("ws", [128, jt], BF16, kind="ExternalInput").ap()
    bias_d = nc.dram_tensor(
        "bias", [128, jt * b_per_core], F32, kind="ExternalInput"
    ).ap()
    ctx_d = nc.dram_tensor("ctx", [b_per_core, 128, jt], F32, kind="ExternalOutput").ap()

    with tile.TileContext(nc) as tc:
        with (
            tc.tile_pool(name="consts", bufs=1) as consts,
            tc.tile_pool(name="xt8p", bufs=8) as xt8p,
            tc.tile_pool(name="xtbp", bufs=6) as xtbp,
            tc.tile_pool(name="scorep", bufs=10) as scorep,
            tc.tile_pool(name="smallp", bufs=2 * n_sc) as smallp,
            tc.tile_pool(name="ebcp", bufs=4) as ebcp,
            tc.tile_pool(name="scrp", bufs=2) as scrp,
            tc.tile_pool(name="ctxp", bufs=4) as ctxp,
            tc.tile_pool(name="ps_main", bufs=4, space="PSUM") as ps_main,
            tc.tile_pool(name="ps_ls", bufs=2, space="PSUM") as ps_ls,
            tc.tile_pool(name="ps_misc", bufs=2, space="PSUM") as ps_misc,
        ):
            # Gate-opening DMA order on the sync ring: the weight slabs for
            # j=0,1 and the first chunk's k-pair slabs go first so matmuls
            # start after ~0.4MB instead of the full 3MB of constants.
            we_sb = consts.tile([128, jt, kt2, 2, 128], FP8)
            nc.sync.dma_start(we_sb[:, 0], we8_d[:, 0])
            nc.sync.dma_start(we_sb[:, 1], we8_d[:, 1])
            xt8_first = xt8p.tile([128, kt2, 2, S_CHUNK], FP8, tag="xt8")
            for t in range(kt2):
                nc.sync.dma_start(
                    xt8_first[:, t], xt8_d[0, 0][:, t * 1024 : (t + 1) * 1024]
                )
            for j in range(2, jt):
                nc.sync.dma_start(we_sb[:, j], we8_d[:, j])
            ws_sb = consts.tile([128, jt], BF16)
            nc.scalar.dma_start(ws_sb[:], ws_d[:])
            bias_sb = consts.tile([128, jt * b_per_core], F32)
            nc.scalar.dma_start(bias_sb[:], bias_d[:])
            ones_bf = consts.tile([1, 128], BF16)
            nc.vector.memset(ones_bf[:], 1.0)
            ones_f32 = consts.tile([1, 128], F32)
            nc.vector.memset(ones_f32[:], 1.0)

            def emit_context_chunk(xtb_bc, ex, ctx4_b, c, tail=False):
                """Broadcast chunk weights, then one fused multiply + one
                fused per-k reduce for the whole chunk.

                The broadcast runs on the otherwise-idle GpSimd engine except
                on the kernel's final chunk, where the PE is idle and the
                ones-matmul + cast path has lower latency.
                """
                ebc = ebcp.tile([128, S_CHUNK], BF16, tag="ebc")
                if tail:
                    bc_ps = ps_misc.tile([128, S_CHUNK], F32, tag="misc")
                    nc.tensor.matmul(
                        bc_ps[:], lhsT=ones_bf[:], rhs=ex[:], start=True, stop=True
                    )
                    nc.vector.tensor_copy(ebc[:], bc_ps[:])
                else:
                    nc.gpsimd.partition_broadcast(ebc[:], ex[:])
                scr = scrp.tile([128, kt, S_CHUNK], BF16, tag="scr")
                ebc_b = ebc[:].unsqueeze(1).broadcast_to((128, kt, S_CHUNK))
                nc.vector.tensor_mul(scr[:], xtb_bc[:], ebc_b)
                nc.vector.reduce_sum(
                    ctx4_b[:, c], scr[:], axis=mybir.AxisListType.X
                )

            def emit_invd(denom_b):
                """softmax denominator -> broadcast 1/d [128, 1]."""
                dsum = smallp.tile([1, 1], F32, tag="dsum")
                nc.vector.reduce_sum(dsum[:], denom_b[:], axis=mybir.AxisListType.X)
                invd = smallp.tile([1, 1], F32, tag="invd")
                nc.vector.reciprocal(invd[:], dsum[:])
                iv_ps = ps_misc.tile([128, S_CHUNK], F32, tag="misc")
                nc.tensor.matmul(
                    iv_ps[:, 0:1], lhsT=ones_f32[:], rhs=invd[:], start=True, stop=True
                )
                invd_bc = smallp.tile([128, 1], F32, tag="invdbc")
                nc.scalar.copy(invd_bc[:], iv_ps[:, 0:1])
                return invd_bc

            def emit_batch_final(b, ctx4_b, invd_bc):
                """Partial reduction over chunks, normalize, store."""
                ctxu = ctxp.tile([128, jt], F32, tag="ctxu")
                nc.vector.reduce_sum(
                    ctxu[:],
                    ctx4_b[:].transpose([0, 2, 1]),
                    axis=mybir.AxisListType.X,
                )
                ctx_b = ctxp.tile([128, jt], F32, tag="ctx")
                nc.vector.tensor_scalar_mul(ctx_b[:], ctxu[:], invd_bc[:])
                nc.sync.dma_start(ctx_d[b], ctx_b[:])

            pending = []  # deferred (context-chunk | invd | batch-final)
            for b in range(b_per_core):
                xt8_tiles = []
                xtb_tiles = []
                for c in range(n_sc):
                    if b == 0 and c == 0:
                        xt8_bc = xt8_first
                    else:
                        xt8_bc = xt8p.tile([128, kt2, 2, S_CHUNK], FP8, tag="xt8")
                        nc.sync.dma_start(xt8_bc[:], xt8_d[b, c])
                    xt8_tiles.append(xt8_bc)
                    xtb_bc = xtbp.tile([128, kt, S_CHUNK], BF16, tag="xtb")
                    nc.scalar.dma_start(xtb_bc[:], xtb_d[b, c])
                    xtb_tiles.append(xtb_bc)

                denom_b = smallp.tile([1, n_sc], F32, tag="denom")
                ctx4_b = ctxp.tile([128, n_sc, kt], F32, tag="ctx4")
                for c in range(n_sc):
                    ls_ps = ps_ls.tile([1, S_CHUNK], F32, tag="ls")
                    score_tiles = []
                    for j in range(jt):
                        mm_ps = ps_main.tile([128, S_CHUNK], F32, tag="main")
                        for t in range(kt2):
                            nc.tensor.matmul(
                                mm_ps[:],
                                lhsT=we_sb[:, j, t],
                                rhs=xt8_tiles[c][:, t],
                                start=(t == 0),
                                stop=(t == kt2 - 1),
                                perf_mode=DR,
                            )
                        sc = scorep.tile([128, S_CHUNK], BF16, tag="score")
                        nc.scalar.activation(
                            sc[:], mm_ps[:], AF.Tanh,
                            bias=bias_sb[:, j * b_per_core + b : j * b_per_core + b + 1],
                            scale=1.0 / WE_SCALE,
                        )
                        score_tiles.append(sc)
                        if j == 0:
                            # deferred work from the previous chunk/batch is
                            # emitted right after the first matmul group, so
                            # the DVE context work overlaps this chunk's
                            # remaining matmul groups
                            for fn in pending:
                                fn()
                            pending = []
                    for j in range(jt):
                        nc.tensor.matmul(
                            ls_ps[:],
                            lhsT=ws_sb[:, j : j + 1],
                            rhs=score_tiles[j][:],
                            start=(j == 0),
                            stop=(j == jt - 1),
                        )
                    ex = smallp.tile([1, S_CHUNK], BF16, tag="exp")
                    nc.scalar.activation(
                        ex[:], ls_ps[:], AF.Exp, accum_out=denom_b[:, c : c + 1]
                    )

                    last_b = b == b_per_core - 1
                    ctx_fn = (
                        lambda xtb_bc=xtb_tiles[c], ex=ex, ctx4_b=ctx4_b, c=c,
                        tl=(last_b and c == n_sc - 1):
                        emit_context_chunk(xtb_bc, ex, ctx4_b, c, tail=tl)
                    )
                    if c < n_sc - 1:
                        pending.append(ctx_fn)
                    elif last_b:
                        # tail of the whole kernel: get 1/d going on the
                        # still-empty DVE queue, then the final context chunk
                        invd_bc = emit_invd(denom_b)
                        ctx_fn()
                        emit_batch_final(b, ctx4_b, invd_bc)
                    else:
                        def batch_tail(ctx_fn=ctx_fn, b=b, ctx4_b=ctx4_b,
                                       denom_b=denom_b):
                            invd_bc = emit_invd(denom_b)
                            ctx_fn()
                            emit_batch_final(b, ctx4_b, invd_bc)
                        pending.append(batch_tail)

    nc.compile()
    return nc


_CACHED = {}


def _get_program(key):
    if key not in _CACHED:
        _CACHED[key] = build_program(*key)
    return _CACHED[key]


def make_in_maps(encoder_out, decoder_hidden_state, We, be, Wd, bd, Ws, bs,
                 b_per_core=B_PER_CORE, s=S, h=H, n_cores=N_CORES):
    kt = h // 128
    kt2 = kt // 2
    jt = h // 128
    n_sc = s // S_CHUNK
    bf = ml_dtypes.bfloat16
    f8 = mybir.dt.np(FP8)

    # we8[p, j, t*256+r*128+m] = 128*We[(2t+r)*128+p, j*128+m]
    we8_a = np.ascontiguousarray(
        (We * WE_SCALE).reshape(kt2, 2, 128, jt, 128).transpose(2, 3, 0, 1, 4)
    ).reshape(128, jt, h).astype(f8)
    ws_a = np.ascontiguousarray(Ws[:, 0].reshape(jt, 128).T).astype(bf)

    dec = decoder_hidden_state[0]  # [32, h]
    bias_all = (be + bd)[None, :] + dec @ Wd  # [32, h] fp32
    in_maps = []
    for i in range(n_cores):
        b0 = i * b_per_core
        xb = encoder_out[b0 : b0 + b_per_core]  # [b, s, h]
        # fp8 PE copy: [b, c, s', t, r, p] -> [b, c, p, t, r, s']
        xt8_a = np.ascontiguousarray(
            xb.reshape(b_per_core, n_sc, S_CHUNK, kt2, 2, 128).transpose(0, 1, 5, 3, 4, 2)
        ).reshape(b_per_core, n_sc, 128, kt2 * 2 * S_CHUNK).astype(f8)
        # bf16 context copy: [b, c, s', k, p] -> [b, c, p, k, s']
        xtb_a = np.ascontiguousarray(
            xb.reshape(b_per_core, n_sc, S_CHUNK, kt, 128).transpose(0, 1, 4, 3, 2)
        ).reshape(b_per_core, n_sc, 128, kt * S_CHUNK).astype(bf)
        bias_a = np.ascontiguousarray(
            bias_all[b0 : b0 + b_per_core].reshape(b_per_core, jt, 128).transpose(2, 1, 0)
        ).reshape(128, jt * b_per_core).astype(np.float32)
        in_maps.append(
            {"xt8": xt8_a, "xtb": xtb_a, "we8": we8_a, "ws": ws_a, "bias": bias_a}
        )
    return in_maps


def kernel(encoder_out, decoder_hidden_state, We, be, Wd, bd, Ws, bs):
    encoder_out = np.asarray(encoder_out, dtype=np.float32)
    decoder_hidden_state = np.asarray(decoder_hidden_state, dtype=np.float32)
    We = np.asarray(We, dtype=np.float32)
    be = np.asarray(be, dtype=np.float32)
    Wd = np.asarray(Wd, dtype=np.float32)
    bd = np.asarray(bd, dtype=np.float32)
    Ws = np.asarray(Ws, dtype=np.float32)
    bs = np.asarray(bs, dtype=np.float32)

    nc = _get_program((B_PER_CORE, S, H))
    in_maps = make_in_maps(
        encoder_out, decoder_hidden_state, We, be, Wd, bd, Ws, bs
    )
    kwargs = {}
    if PROFILE["trace"]:
        kwargs = {"trace": True, "tmpdir": PROFILE["tmpdir"]}
    res = run_bass_kernel_spmd(nc, in_maps, list(range(N_CORES)), **kwargs)
    PROFILE["last_result"] = res

    out = np.empty((N_CORES * B_PER_CORE, H), dtype=np.float32)
    for i in range(N_CORES):
        ctx = res.results[i]["ctx"]  # [b, 128, jt]
        out[i * B_PER_CORE : (i + 1) * B_PER_CORE] = (
            ctx.transpose(0, 2, 1).reshape(B_PER_CORE, H)
        )
    return out


# revision 7
# speedup vs baseline: 1.4143x; 1.0043x over previous
"""Bahdanau attention fused kernel for Trainium2, 8-core data-parallel.

Reference computation (per batch b of 32, H=1024, S=2048):
    enc_score = encoder_out @ We + be                    [B, S, H]
    dec_score = dec @ Wd + bd                            [B, 1, H]
    score     = tanh(enc_score + dec_score)              [B, S, H]
    ls        = score @ Ws + bs                          [B, S, 1]
    w         = softmax(ls, axis=S)
    out       = sum_s w[b,s] * encoder_out[b,s,:]        [B, H]

Sharding: batch 32 -> 4 per core across 8 cores; weights replicated.
The tiny dec-score GEMM is folded into the host-side bias preparation:
bias[b] = be + bd + dec[b] @ Wd. bs is dropped (softmax shift-invariant).

The main GEMM (enc_score) runs in fp8-e4m3 with MatmulPerfMode.DoubleRow:
the PE processes two 128-deep k-tiles per matmul (2 fp8 weights/cell), so
the 1024-deep contraction takes 4 matmuls instead of 8. We is pre-scaled
by 128 host-side to keep its +-1/32 values out of fp8 subnormals; the
tanh activation applies scale=1/128 to compensate. X is shipped twice:
fp8 (PE main GEMM) and bf16 (VectorE context path; fp8 would cost ~2.4%
relative error on the context).

Per-core device layout (prepared host-side):
    xt8  [4, 4, 128, 4096] fp8  xt8[b,c,p,t*1024+r*512+s'] = X[b, c*512+s', (2t+r)*128+p]
    xtb  [4, 4, 128, 4096] bf16 xtb[b,c,p,k*512+s']        = X[b, c*512+s', k*128+p]
    we8  [128, 8, 1024]    fp8  we8[p,j,t*256+r*128+m]     = 128*We[(2t+r)*128+p, j*128+m]
    ws   [128, 8]          bf16 ws[p, j]                   = Ws[j*128+p, 0]
    bias [128, 32]         f32  bias[p, j*4+b]             = (be+bd+dec[b]@Wd)[j*128+p]
    out: ctx [4, 128, 8]   f32  ctx[b, p, j]               = out[b, j*128+p]

Device schedule per (batch, 512-wide s-chunk):
  - 8 j-groups x 4 DoubleRow matmuls accumulate enc_score.T in PSUM;
    ScalarE evacuates with fused tanh(psum/128 + bias) -> bf16 scores
  - ls.T = sum_j Ws[j].T @ score_j in PSUM [1,512] (bf16 matmuls)
  - ScalarE exp with fused accum_out denominator
  - context: exp weights broadcast to 128 partitions (GpSimd; PE
    ones-matmul on the kernel tail), then ONE fused VectorE multiply
    [128,8,512] (broadcast-AP weights) + ONE fused reduce -> [128,8]
    per-k partials; deferred one chunk so the PE never waits
  - per batch: softmax denominator reciprocal, partial-sum over chunks,
    normalize, store

Startup: the first xt8 chunk (512KB contiguous) goes on the sync DMA
ring while the j-major weight slabs (128KB contiguous each) go on the
scalar ring in parallel; the first matmul gates on xt8[0,0] + we[j0].
All steady-state chunk DMAs ride the sync ring so the scalar queue is
dedicated to the tanh/exp activation chain.
"""

import numpy as np
import ml_dtypes

import concourse.tile as tile
from concourse import bacc, mybir
from concourse.bass_utils import run_bass_kernel_spmd

BF16 = mybir.dt.bfloat16
F32 = mybir.dt.float32
FP8 = mybir.dt.float8e4
AF = mybir.ActivationFunctionType
DR = mybir.MatmulPerfMode.DoubleRow

N_CORES = 8
H = 1024
S = 2048
B_PER_CORE = 4
S_CHUNK = 512
WE_SCALE = 128.0

# test.py can flip this to get a profiled run; the grading path never does.
PROFILE = {"trace": False, "tmpdir": None}


def build_program(b_per_core=B_PER_CORE, s=S, h=H):
    kt = h // 128          # 8  bf16 k-tiles (context path)
    kt2 = kt // 2          # 4  fp8 DoubleRow k-pair tiles
    jt = h // 128          # 8  output h tiles
    n_sc = s // S_CHUNK    # 4  s chunks
    nc = bacc.Bacc("TRN2", target_bir_lowering=False, debug=False)

    xt8_d = nc.dram_tensor(
        "xt8", [b_per_core, n_sc, 128, kt2 * 2 * S_CHUNK], FP8, kind="ExternalInput"
    ).ap()
    xtb_d = nc.dram_tensor(
        "xtb", [b_per_core, n_sc, 128, kt * S_CHUNK], BF16, kind="ExternalInput"
    ).ap()
    # j-major so each per-j weight slab is one contiguous 128KB transfer
    we8_d = nc.dram_tensor("we8", [jt, 128, h], FP8, kind="ExternalInput").ap()
    ws_d = nc.dram_tensor("ws", [128, jt], BF16, kind="ExternalInput").ap()
    bias_d = nc.dram_tensor(
        "bias", [128, jt * b_per_core], F32, kind="ExternalInput"
    ).ap()
    ctx_d = nc.dram_tensor("ctx", [b_per_core, 128, jt], F32, kind="ExternalOutput").ap()

    with tile.TileContext(nc) as tc:
        with (
            tc.tile_pool(name="consts", bufs=1) as consts,
            tc.tile_pool(name="xt8p", bufs=8) as xt8p,
            tc.tile_pool(name="xtbp", bufs=6) as xtbp,
            tc.tile_pool(name="scorep", bufs=10) as scorep,
            tc.tile_pool(name="smallp", bufs=2 * n_sc) as smallp,
            tc.tile_pool(name="ebcp", bufs=4) as ebcp,
            tc.tile_pool(name="scrp", bufs=2) as scrp,
            tc.tile_pool(name="ctxp", bufs=4) as ctxp,
            tc.tile_pool(name="ps_main", bufs=4, space="PSUM") as ps_main,
            tc.tile_pool(name="ps_ls", bufs=2, space="PSUM") as ps_ls,
            tc.tile_pool(name="ps_misc", bufs=2, space="PSUM") as ps_misc,
        ):
            # Gate-opening DMAs run on BOTH rings in parallel: the first
            # chunk (one contiguous 512KB) on sync, the weight slabs
            # (contiguous 128KB each, j-major) on scalar. The first matmul
            # gates on xt8[0,0] + we[j=0] only.
            xt8_first = xt8p.tile([128, kt2, 2, S_CHUNK], FP8, tag="xt8")
            nc.sync.dma_start(xt8_first[:], xt8_d[0, 0])
            we_sb = consts.tile([128, jt, kt2, 2, 128], FP8)
            for j in range(jt):
                nc.scalar.dma_start(we_sb[:, j], we8_d[j])
            ws_sb = consts.tile([128, jt], BF16)
            nc.scalar.dma_start(ws_sb[:], ws_d[:])
            bias_sb = consts.tile([128, jt * b_per_core], F32)
            nc.scalar.dma_start(bias_sb[:], bias_d[:])
            ones_bf = consts.tile([1, 128], BF16)
            nc.vector.memset(ones_bf[:], 1.0)
            ones_f32 = consts.tile([1, 128], F32)
            nc.vector.memset(ones_f32[:], 1.0)

            def emit_context_chunk(xtb_bc, ex, ctx4_b, c, tail=False):
                """Broadcast chunk weights, then one fused multiply + one
                fused per-k reduce for the whole chunk.

                The broadcast runs on the otherwise-idle GpSimd engine except
                on the kernel's final chunk, where the PE is idle and the
                ones-matmul + cast path has lower latency.
                """
                ebc = ebcp.tile([128, S_CHUNK], BF16, tag="ebc")
                if tail:
                    bc_ps = ps_misc.tile([128, S_CHUNK], F32, tag="misc")
                    nc.tensor.matmul(
                        bc_ps[:], lhsT=ones_bf[:], rhs=ex[:], start=True, stop=True
                    )
                    nc.vector.tensor_copy(ebc[:], bc_ps[:])
                else:
                    nc.gpsimd.partition_broadcast(ebc[:], ex[:])
                scr = scrp.tile([128, kt, S_CHUNK], BF16, tag="scr")
                ebc_b = ebc[:].unsqueeze(1).broadcast_to((128, kt, S_CHUNK))
                nc.vector.tensor_mul(scr[:], xtb_bc[:], ebc_b)
                nc.vector.reduce_sum(
                    ctx4_b[:, c], scr[:], axis=mybir.AxisListType.X
                )

            def emit_invd(denom_b):
                """softmax denominator -> broadcast 1/d [128, 1]."""
                dsum = smallp.tile([1, 1], F32, tag="dsum")
                nc.vector.reduce_sum(dsum[:], denom_b[:], axis=mybir.AxisListType.X)
                invd = smallp.tile([1, 1], F32, tag="invd")
                nc.vector.reciprocal(invd[:], dsum[:])
                iv_ps = ps_misc.tile([128, S_CHUNK], F32, tag="misc")
                nc.tensor.matmul(
                    iv_ps[:, 0:1], lhsT=ones_f32[:], rhs=invd[:], start=True, stop=True
                )
                invd_bc = smallp.tile([128, 1], F32, tag="invdbc")
                nc.scalar.copy(invd_bc[:], iv_ps[:, 0:1])
                return invd_bc

            def emit_batch_final(b, ctx4_b, invd_bc):
                """Partial reduction over chunks, normalize, store."""
                ctxu = ctxp.tile([128, jt], F32, tag="ctxu")
                nc.vector.reduce_sum(
                    ctxu[:],
                    ctx4_b[:].transpose([0, 2, 1]),
                    axis=mybir.AxisListType.X,
                )
                ctx_b = ctxp.tile([128, jt], F32, tag="ctx")
                nc.vector.tensor_scalar_mul(ctx_b[:], ctxu[:], invd_bc[:])
                nc.sync.dma_start(ctx_d[b], ctx_b[:])

            pending = []  # deferred (context-chunk | invd | batch-final)
            for b in range(b_per_core):
                xt8_tiles = []
                xtb_tiles = []
                for c in range(n_sc):
                    if b == 0 and c == 0:
                        xt8_bc = xt8_first
                    else:
                        xt8_bc = xt8p.tile([128, kt2, 2, S_CHUNK], FP8, tag="xt8")
                        nc.sync.dma_start(xt8_bc[:], xt8_d[b, c])
                    xt8_tiles.append(xt8_bc)
                    # xtb rides the sync ring too: DMA issues on the scalar
                    # queue would steal ~1.8us/chunk from the tanh ACT chain
                    # that the ls matmuls gate on.
                    xtb_bc = xtbp.tile([128, kt, S_CHUNK], BF16, tag="xtb")
                    nc.sync.dma_start(xtb_bc[:], xtb_d[b, c])
                    xtb_tiles.append(xtb_bc)

                denom_b = smallp.tile([1, n_sc], F32, tag="denom")
                ctx4_b = ctxp.tile([128, n_sc, kt], F32, tag="ctx4")
                for c in range(n_sc):
                    ls_ps = ps_ls.tile([1, S_CHUNK], F32, tag="ls")
                    score_tiles = []
                    for j in range(jt):
                        mm_ps = ps_main.tile([128, S_CHUNK], F32, tag="main")
                        for t in range(kt2):
                            nc.tensor.matmul(
                                mm_ps[:],
                                lhsT=we_sb[:, j, t],
                                rhs=xt8_tiles[c][:, t],
                                start=(t == 0),
                                stop=(t == kt2 - 1),
                                perf_mode=DR,
                            )
                        sc = scorep.tile([128, S_CHUNK], BF16, tag="score")
                        nc.scalar.activation(
                            sc[:], mm_ps[:], AF.Tanh,
                            bias=bias_sb[:, j * b_per_core + b : j * b_per_core + b + 1],
                            scale=1.0 / WE_SCALE,
                        )
                        score_tiles.append(sc)
                        if j == 0:
                            # deferred work from the previous chunk/batch is
                            # emitted right after the first matmul group, so
                            # the DVE context work overlaps this chunk's
                            # remaining matmul groups
                            for fn in pending:
                                fn()
                            pending = []
                    for j in range(jt):
                        nc.tensor.matmul(
                            ls_ps[:],
                            lhsT=ws_sb[:, j : j + 1],
                            rhs=score_tiles[j][:],
                            start=(j == 0),
                            stop=(j == jt - 1),
                        )
                    ex = smallp.tile([1, S_CHUNK], BF16, tag="exp")
                    nc.scalar.activation(
                        ex[:], ls_ps[:], AF.Exp, accum_out=denom_b[:, c : c + 1]
                    )

                    last_b = b == b_per_core - 1
                    ctx_fn = (
                        lambda xtb_bc=xtb_tiles[c], ex=ex, ctx4_b=ctx4_b, c=c,
                        tl=(last_b and c == n_sc - 1):
                        emit_context_chunk(xtb_bc, ex, ctx4_b, c, tail=tl)
                    )
                    if c < n_sc - 1:
                        pending.append(ctx_fn)
                    elif last_b:
                        # tail of the whole kernel: get 1/d going on the
                        # still-empty DVE queue, then the final context chunk
                        invd_bc = emit_invd(denom_b)
                        ctx_fn()
                        emit_batch_final(b, ctx4_b, invd_bc)
                    else:
                        def batch_tail(ctx_fn=ctx_fn, b=b, ctx4_b=ctx4_b,
                                       denom_b=denom_b):
                            invd_bc = emit_invd(denom_b)
                            ctx_fn()
                            emit_batch_final(b, ctx4_b, invd_bc)
                        pending.append(batch_tail)

    nc.compile()
    return nc


_CACHED = {}


def _get_program(key):
    if key not in _CACHED:
        _CACHED[key] = build_program(*key)
    return _CACHED[key]


def make_in_maps(encoder_out, decoder_hidden_state, We, be, Wd, bd, Ws, bs,
                 b_per_core=B_PER_CORE, s=S, h=H, n_cores=N_CORES):
    kt = h // 128
    kt2 = kt // 2
    jt = h // 128
    n_sc = s // S_CHUNK
    bf = ml_dtypes.bfloat16
    f8 = mybir.dt.np(FP8)

    # we8[j, p, t*256+r*128+m] = 128*We[(2t+r)*128+p, j*128+m]
    we8_a = np.ascontiguousarray(
        (We * WE_SCALE).reshape(kt2, 2, 128, jt, 128).transpose(3, 2, 0, 1, 4)
    ).reshape(jt, 128, h).astype(f8)
    ws_a = np.ascontiguousarray(Ws[:, 0].reshape(jt, 128).T).astype(bf)

    dec = decoder_hidden_state[0]  # [32, h]
    bias_all = (be + bd)[None, :] + dec @ Wd  # [32, h] fp32
    in_maps = []
    for i in range(n_cores):
        b0 = i * b_per_core
        xb = encoder_out[b0 : b0 + b_per_core]  # [b, s, h]
        # fp8 PE copy: [b, c, s', t, r, p] -> [b, c, p, t, r, s']
        xt8_a = np.ascontiguousarray(
            xb.reshape(b_per_core, n_sc, S_CHUNK, kt2, 2, 128).transpose(0, 1, 5, 3, 4, 2)
        ).reshape(b_per_core, n_sc, 128, kt2 * 2 * S_CHUNK).astype(f8)
        # bf16 context copy: [b, c, s', k, p] -> [b, c, p, k, s']
        xtb_a = np.ascontiguousarray(
            xb.reshape(b_per_core, n_sc, S_CHUNK, kt, 128).transpose(0, 1, 4, 3, 2)
        ).reshape(b_per_core, n_sc, 128, kt * S_CHUNK).astype(bf)
        bias_a = np.ascontiguousarray(
            bias_all[b0 : b0 + b_per_core].reshape(b_per_core, jt, 128).transpose(2, 1, 0)
        ).reshape(128, jt * b_per_core).astype(np.float32)
        in_maps.append(
            {"xt8": xt8_a, "xtb": xtb_a, "we8": we8_a, "ws": ws_a, "bias": bias_a}
        )
    return in_maps


def kernel(encoder_out, decoder_hidden_state, We, be, Wd, bd, Ws, bs):
    encoder_out = np.asarray(encoder_out, dtype=np.float32)
    decoder_hidden_state = np.asarray(decoder_hidden_state, dtype=np.float32)
    We = np.asarray(We, dtype=np.float32)
    be = np.asarray(be, dtype=np.float32)
    Wd = np.asarray(Wd, dtype=np.float32)
    bd = np.asarray(bd, dtype=np.float32)
    Ws = np.asarray(Ws, dtype=np.float32)
    bs = np.asarray(bs, dtype=np.float32)

    nc = _get_program((B_PER_CORE, S, H))
    in_maps = make_in_maps(
        encoder_out, decoder_hidden_state, We, be, Wd, bd, Ws, bs
    )
    kwargs = {}
    if PROFILE["trace"]:
        kwargs = {"trace": True, "tmpdir": PROFILE["tmpdir"]}
    res = run_bass_kernel_spmd(nc, in_maps, list(range(N_CORES)), **kwargs)
    PROFILE["last_result"] = res

    out = np.empty((N_CORES * B_PER_CORE, H), dtype=np.float32)
    for i in range(N_CORES):
        ctx = res.results[i]["ctx"]  # [b, 128, jt]
        out[i * B_PER_CORE : (i + 1) * B_PER_CORE] = (
            ctx.transpose(0, 2, 1).reshape(B_PER_CORE, H)
        )
    return out


# revision 12
# speedup vs baseline: 1.4660x; 1.0366x over previous
"""Bahdanau attention fused kernel for Trainium2, 8-core data-parallel.

Reference computation (per batch b of 32, H=1024, S=2048):
    enc_score = encoder_out @ We + be                    [B, S, H]
    dec_score = dec @ Wd + bd                            [B, 1, H]
    score     = tanh(enc_score + dec_score)              [B, S, H]
    ls        = score @ Ws + bs                          [B, S, 1]
    w         = softmax(ls, axis=S)
    out       = sum_s w[b,s] * encoder_out[b,s,:]        [B, H]

Sharding: batch 32 -> 4 per core across 8 cores; weights replicated.
The tiny dec-score GEMM is folded into the host-side bias preparation:
bias[b] = be + bd + dec[b] @ Wd. bs is dropped (softmax shift-invariant).

Numerics: the main GEMM and the ls projection run in fp8-e4m3 with
MatmulPerfMode.DoubleRow (two 128-deep k-tiles per matmul). We and Ws
are pre-scaled by 128 host-side to clear fp8 subnormals; the tanh/exp
activations apply scale=1/128 to compensate. X is shipped twice: fp8
(PE) and bf16 (VectorE context path). Total rel err ~1.7e-2 (sim-
verified; hardware matches the numpy fp8 sim to ~1e-5).

Per-core device layout (prepared host-side):
    xt8  [4, 4, 128, 4096] fp8  xt8[b,c,p,t*1024+r*512+s'] = X[b, c*512+s', (2t+r)*128+p]
    xtb  [4, 4, 128, 4096] bf16 xtb[b,c,p,k*512+s']        = X[b, c*512+s', k*128+p]
    we8  [8, 128, 1024]    fp8  we8[j,p,t*256+r*128+m]     = 128*We[(2t+r)*128+p, j*128+m]
    ws8  [128, 8]          fp8  ws8[p,t*2+r]               = 128*Ws[(2t+r)*128+p, 0]
    bias [128, 32]         f32  bias[p, j*4+b]             = (be+bd+dec[b]@Wd)[j*128+p]
    xs3  [4, 128, 1024]    bf16 xs3[si,p,n] = X[3, 3*512+si*128+p, n]   (tail chunk, s-major)
Outputs:
    ctx  [4, 128, 8] f32: batches 0-2 normalized contexts; batch 3 the
         UNNORMALIZED partial over chunks 0-2
    ctx3 [1, 1024]  f32: batch 3's unnormalized chunk-3 partial (h on free)
    den3 [1, 1]     f32: batch 3's softmax denominator
    (host: out[3] = (ctx[3].T + ctx3) / den3)

Device schedule per (batch, 512-wide s-chunk):
  - 8 j-groups x 4 DoubleRow matmuls accumulate enc_score.T in PSUM;
    ScalarE evacuates with fused tanh(psum/128 + bias) -> fp8 score
    pairs (j even/odd interleaved slabs for the ls DoubleRow rhs)
  - ls.T = 4 DoubleRow matmuls over score pairs -> PSUM [1,512]
  - the exp + context work for chunk c is DEFERRED into chunk c+1's
    matmul phase: the in-order ScalarE queue would otherwise stall on
    exp (which waits for the ls chain) ahead of the next chunk's tanh
    evacuations that the next ls matmuls gate on
  - context: exp weights broadcast to 128 partitions (GpSimd), one
    fused VectorE multiply [128,8,512] (broadcast-AP) + one fused
    per-k reduce -> ctx partials
  - kernel tail (last batch, last chunk): the context runs on the
    otherwise-idle PE instead of the DVE: exp halves -> PE transposes
    to [128,4] -> 8 matmuls against the s-major bf16 chunk -> [1,1024]
    unnormalized context, merged with the denominator on the host
"""

import numpy as np
import ml_dtypes

import concourse.tile as tile
from concourse import bacc, mybir
from concourse.bass_utils import run_bass_kernel_spmd

BF16 = mybir.dt.bfloat16
F32 = mybir.dt.float32
FP8 = mybir.dt.float8e4
AF = mybir.ActivationFunctionType
DR = mybir.MatmulPerfMode.DoubleRow

N_CORES = 8
H = 1024
S = 2048
B_PER_CORE = 4
S_CHUNK = 512
WE_SCALE = 128.0

# test.py can flip this to get a profiled run; the grading path never does.
PROFILE = {"trace": False, "tmpdir": None}


def build_program(b_per_core=B_PER_CORE, s=S, h=H):
    kt = h // 128          # 8  bf16 k-tiles (context path)
    kt2 = kt // 2          # 4  fp8 DoubleRow k-pair tiles
    jt = h // 128          # 8  output h tiles
    jt2 = jt // 2          # 4  score pair tiles (ls DoubleRow)
    n_sc = s // S_CHUNK    # 4  s chunks
    nc = bacc.Bacc("TRN2", target_bir_lowering=False, debug=False)

    xt8_d = nc.dram_tensor(
        "xt8", [b_per_core, n_sc, 128, kt2 * 2 * S_CHUNK], FP8, kind="ExternalInput"
    ).ap()
    xtb_d = nc.dram_tensor(
        "xtb", [b_per_core, n_sc, 128, kt * S_CHUNK], BF16, kind="ExternalInput"
    ).ap()
    # j-major so each per-j weight slab is one contiguous 128KB transfer
    we8_d = nc.dram_tensor("we8", [jt, 128, h], FP8, kind="ExternalInput").ap()
    # ws pair slabs padded to 16 bytes: DoubleRow ldweights requires the
    # k-pair step to be a multiple of 16 bytes
    ws8_d = nc.dram_tensor("ws8", [128, jt2 * 2 * 16], FP8, kind="ExternalInput").ap()
    bias_d = nc.dram_tensor(
        "bias", [128, jt * b_per_core], F32, kind="ExternalInput"
    ).ap()
    xs3_d = nc.dram_tensor("xs3", [S_CHUNK // 128, 128, h], BF16, kind="ExternalInput").ap()
    ctx_d = nc.dram_tensor("ctx", [b_per_core, 128, jt], F32, kind="ExternalOutput").ap()
    ctx3_d = nc.dram_tensor("ctx3", [1, h], F32, kind="ExternalOutput").ap()
    den3_d = nc.dram_tensor("den3", [1, 1], F32, kind="ExternalOutput").ap()

    with tile.TileContext(nc) as tc:
        with (
            tc.tile_pool(name="consts", bufs=1) as consts,
            tc.tile_pool(name="xt8p", bufs=8) as xt8p,
            tc.tile_pool(name="xtbp", bufs=6) as xtbp,
            tc.tile_pool(name="scorep", bufs=6) as scorep,
            tc.tile_pool(name="smallp", bufs=2 * n_sc) as smallp,
            tc.tile_pool(name="ebcp", bufs=4) as ebcp,
            tc.tile_pool(name="scrp", bufs=2) as scrp,
            tc.tile_pool(name="ctxp", bufs=4) as ctxp,
            tc.tile_pool(name="ps_main", bufs=4, space="PSUM") as ps_main,
            tc.tile_pool(name="ps_ls", bufs=2, space="PSUM") as ps_ls,
            tc.tile_pool(name="ps_misc", bufs=1, space="PSUM") as ps_misc,
        ):
            # Gate-opening DMAs run on BOTH rings in parallel: the first
            # chunk (one contiguous 512KB) on sync, the weight slabs
            # (contiguous 128KB each, j-major) on scalar. The first matmul
            # gates on xt8[0,0] + we[j=0] only.
            xt8_first = xt8p.tile([128, kt2, 2, S_CHUNK], FP8, tag="xt8")
            nc.sync.dma_start(xt8_first[:], xt8_d[0, 0])
            we_sb = consts.tile([128, jt, kt2, 2, 128], FP8)
            for j in range(jt):
                nc.scalar.dma_start(we_sb[:, j], we8_d[j])
            ws_sb = consts.tile([128, jt2, 2, 16], FP8)
            nc.scalar.dma_start(ws_sb[:], ws8_d[:])
            bias_sb = consts.tile([128, jt * b_per_core], F32)
            nc.scalar.dma_start(bias_sb[:], bias_d[:])
            xs_sb = consts.tile([128, S_CHUNK // 128, h], BF16)
            for si in range(S_CHUNK // 128):
                nc.sync.dma_start(xs_sb[:, si], xs3_d[si])
            ones_bf = consts.tile([1, 128], BF16)
            nc.vector.memset(ones_bf[:], 1.0)
            ones_f32 = consts.tile([1, 128], F32)
            nc.vector.memset(ones_f32[:], 1.0)

            def emit_exp(ls_ps, denom_b, c):
                """exp(ls/128) -> bf16 weights + f32 denominator slot."""
                ex = smallp.tile([1, S_CHUNK], BF16, tag="exp")
                nc.scalar.activation(
                    ex[:], ls_ps[:], AF.Exp, scale=1.0 / WE_SCALE,
                    accum_out=denom_b[:, c : c + 1],
                )
                return ex

            def emit_context_chunk(xtb_bc, ex, ctx4_b, c):
                """Broadcast chunk weights (GpSimd), then one fused multiply
                + one fused per-k reduce for the whole chunk (DVE)."""
                ebc = ebcp.tile([128, S_CHUNK], BF16, tag="ebc")
                nc.gpsimd.partition_broadcast(ebc[:], ex[:])
                scr = scrp.tile([128, kt, S_CHUNK], BF16, tag="scr")
                ebc_b = ebc[:].unsqueeze(1).broadcast_to((128, kt, S_CHUNK))
                nc.vector.tensor_mul(scr[:], xtb_bc[:], ebc_b)
                nc.vector.reduce_sum(
                    ctx4_b[:, c], scr[:], axis=mybir.AxisListType.X
                )

            def emit_invd(denom_b, width):
                """softmax denominator -> broadcast 1/d [128, 1]."""
                dsum = smallp.tile([1, 1], F32, tag="dsum")
                nc.vector.reduce_sum(
                    dsum[:], denom_b[:, :width], axis=mybir.AxisListType.X
                )
                invd = smallp.tile([1, 1], F32, tag="invd")
                nc.vector.reciprocal(invd[:], dsum[:])
                iv_ps = ps_misc.tile([128, S_CHUNK], F32, tag="misc")
                nc.tensor.matmul(
                    iv_ps[:, 0:1], lhsT=ones_f32[:], rhs=invd[:], start=True, stop=True
                )
                invd_bc = smallp.tile([128, 1], F32, tag="invdbc")
                nc.scalar.copy(invd_bc[:], iv_ps[:, 0:1])
                return invd_bc

            def emit_batch_final(b, ctx4_b, invd_bc, width):
                """Partial reduction over chunks, normalize, store."""
                ctxu = ctxp.tile([128, jt], F32, tag="ctxu")
                nc.vector.reduce_sum(
                    ctxu[:],
                    ctx4_b[:, :width].transpose([0, 2, 1]),
                    axis=mybir.AxisListType.X,
                )
                if invd_bc is None:
                    nc.sync.dma_start(ctx_d[b], ctxu[:])
                else:
                    ctx_b = ctxp.tile([128, jt], F32, tag="ctx")
                    nc.vector.tensor_scalar_mul(ctx_b[:], ctxu[:], invd_bc[:])
                    nc.sync.dma_start(ctx_d[b], ctx_b[:])

            def emit_pe_tail(ls_ps, denom_b):
                """Kernel tail: chunk context on the idle PE.

                exp in two halves -> PE transposes into exT [128,4] ->
                8 matmuls against the s-major bf16 chunk -> unnormalized
                ctx3 [1,1024]; denominator shipped separately for the
                host-side divide.
                """
                # [128, 4, 2] with writes to [:, si, 0]: PSUM matmul writes
                # must be 4-byte aligned, so the bf16 columns sit at stride 4
                exT_ps = ps_misc.tile([128, 4, 2], BF16, tag="exT")
                ex_h = []
                for hh in range(2):
                    exh = smallp.tile([1, 256], BF16, tag="exph")
                    nc.scalar.activation(
                        exh[:], ls_ps[:, hh * 256 : (hh + 1) * 256], AF.Exp,
                        scale=1.0 / WE_SCALE,
                        accum_out=denom_b[:, n_sc - 1 + hh : n_sc + hh],
                    )
                    ex_h.append(exh)
                for si in range(4):
                    nc.tensor.transpose(
                        exT_ps[:, si, 0:1],
                        ex_h[si // 2][:, (si % 2) * 128 : (si % 2 + 1) * 128],
                        ones_bf[:, 0:1],
                    )
                exT = smallp.tile([128, 4], BF16, tag="exT_sb")
                nc.scalar.copy(exT[:], exT_ps[:, :, 0])
                ctx3_sb = ctxp.tile([1, h], F32, tag="ctx3")
                for hh in range(2):
                    c3_ps = ps_ls.tile([1, S_CHUNK], F32, tag="ls")
                    for si in range(4):
                        nc.tensor.matmul(
                            c3_ps[:],
                            lhsT=exT[:, si : si + 1],
                            rhs=xs_sb[:, si, hh * S_CHUNK : (hh + 1) * S_CHUNK],
                            start=(si == 0),
                            stop=(si == 3),
                        )
                    if hh == 0:
                        nc.scalar.copy(ctx3_sb[:, :S_CHUNK], c3_ps[:])
                    else:
                        nc.vector.tensor_copy(ctx3_sb[:, S_CHUNK:], c3_ps[:])
                # denominator: all 4 chunks (last chunk in 2 half slots)
                dsum = smallp.tile([1, 1], F32, tag="dsum")
                nc.vector.reduce_sum(
                    dsum[:], denom_b[:], axis=mybir.AxisListType.X
                )
                nc.sync.dma_start(ctx3_d[:], ctx3_sb[:])
                nc.sync.dma_start(den3_d[:], dsum[:])

            pending = []  # deferred (exp | context-chunk | invd | batch-final)
            for b in range(b_per_core):
                last_b = b == b_per_core - 1
                xt8_tiles = []
                xtb_tiles = []
                for c in range(n_sc):
                    if b == 0 and c == 0:
                        xt8_bc = xt8_first
                    else:
                        xt8_bc = xt8p.tile([128, kt2, 2, S_CHUNK], FP8, tag="xt8")
                        nc.sync.dma_start(xt8_bc[:], xt8_d[b, c])
                    xt8_tiles.append(xt8_bc)
                    # xtb rides the sync ring too: DMA issues on the scalar
                    # queue would steal ~1.8us/chunk from the tanh ACT chain
                    # that the ls matmuls gate on.
                    xtb_bc = xtbp.tile([128, kt, S_CHUNK], BF16, tag="xtb")
                    nc.sync.dma_start(xtb_bc[:], xtb_d[b, c])
                    xtb_tiles.append(xtb_bc)

                denom_b = smallp.tile([1, n_sc + 1], F32, tag="denom")
                ctx4_b = ctxp.tile([128, n_sc, kt], F32, tag="ctx4")
                for c in range(n_sc):
                    ls_ps = ps_ls.tile([1, S_CHUNK], F32, tag="ls")
                    score_tiles = []
                    for j in range(jt):
                        mm_ps = ps_main.tile([128, S_CHUNK], F32, tag="main")
                        for t in range(kt2):
                            nc.tensor.matmul(
                                mm_ps[:],
                                lhsT=we_sb[:, j, t],
                                rhs=xt8_tiles[c][:, t],
                                start=(t == 0),
                                stop=(t == kt2 - 1),
                                perf_mode=DR,
                            )
                        if j % 2 == 0:
                            scp = scorep.tile([128, 2, S_CHUNK], FP8, tag="score")
                            score_tiles.append(scp)
                        nc.scalar.activation(
                            score_tiles[j // 2][:, j % 2], mm_ps[:], AF.Tanh,
                            bias=bias_sb[:, j * b_per_core + b : j * b_per_core + b + 1],
                            scale=1.0 / WE_SCALE,
                        )
                        if j == 0:
                            # deferred work from the previous chunk/batch is
                            # emitted right after the first matmul group, so
                            # its ScalarE exp lands behind this chunk's first
                            # tanh and the DVE context work overlaps this
                            # chunk's remaining matmul groups
                            for fn in pending:
                                fn()
                            pending = []
                    for tj in range(jt2):
                        nc.tensor.matmul(
                            ls_ps[:],
                            lhsT=ws_sb[:, tj, :, 0:1],
                            rhs=score_tiles[tj][:],
                            start=(tj == 0),
                            stop=(tj == jt2 - 1),
                            perf_mode=DR,
                        )

                    if last_b and c == n_sc - 1:
                        # kernel tail: denominator + context via the idle PE
                        emit_pe_tail(ls_ps, denom_b)
                        emit_batch_final(b, ctx4_b, None, n_sc - 1)
                    elif c == n_sc - 1:
                        def batch_tail(ls_ps=ls_ps, b=b, c=c, ctx4_b=ctx4_b,
                                       denom_b=denom_b, xtb_bc=xtb_tiles[c]):
                            ex = emit_exp(ls_ps, denom_b, c)
                            invd_bc = emit_invd(denom_b, n_sc)
                            emit_context_chunk(xtb_bc, ex, ctx4_b, c)
                            emit_batch_final(b, ctx4_b, invd_bc, n_sc)
                        pending.append(batch_tail)
                    else:
                        def ctx_fn(ls_ps=ls_ps, c=c, ctx4_b=ctx4_b,
                                   denom_b=denom_b, xtb_bc=xtb_tiles[c]):
                            ex = emit_exp(ls_ps, denom_b, c)
                            emit_context_chunk(xtb_bc, ex, ctx4_b, c)
                        pending.append(ctx_fn)

    nc.compile()
    return nc


_CACHED = {}


def _get_program(key):
    if key not in _CACHED:
        _CACHED[key] = build_program(*key)
    return _CACHED[key]


def make_in_maps(encoder_out, decoder_hidden_state, We, be, Wd, bd, Ws, bs,
                 b_per_core=B_PER_CORE, s=S, h=H, n_cores=N_CORES):
    kt = h // 128
    kt2 = kt // 2
    jt = h // 128
    n_sc = s // S_CHUNK
    bf = ml_dtypes.bfloat16
    f8 = mybir.dt.np(FP8)

    # we8[j, p, t*256+r*128+m] = 128*We[(2t+r)*128+p, j*128+m]
    we8_a = np.ascontiguousarray(
        (We * WE_SCALE).reshape(kt2, 2, 128, jt, 128).transpose(3, 2, 0, 1, 4)
    ).reshape(jt, 128, h).astype(f8)
    # ws8[p, (t*2+r)*16] = 128*Ws[(2t+r)*128+p, 0]; 16-byte padded pair slabs
    ws8_a = np.zeros((128, kt2, 2, 16), dtype=f8)
    ws8_a[:, :, :, 0] = (
        (Ws[:, 0] * WE_SCALE).reshape(kt2, 2, 128).transpose(2, 0, 1).astype(f8)
    )
    ws8_a = ws8_a.reshape(128, kt2 * 2 * 16)

    dec = decoder_hidden_state[0]  # [32, h]
    bias_all = (be + bd)[None, :] + dec @ Wd  # [32, h] fp32
    in_maps = []
    for i in range(n_cores):
        b0 = i * b_per_core
        xb = encoder_out[b0 : b0 + b_per_core]  # [b, s, h]
        # fp8 PE copy: [b, c, s', t, r, p] -> [b, c, p, t, r, s']
        xt8_a = np.ascontiguousarray(
            xb.reshape(b_per_core, n_sc, S_CHUNK, kt2, 2, 128).transpose(0, 1, 5, 3, 4, 2)
        ).reshape(b_per_core, n_sc, 128, kt2 * 2 * S_CHUNK).astype(f8)
        # bf16 context copy: [b, c, s', k, p] -> [b, c, p, k, s']
        xtb_a = np.ascontiguousarray(
            xb.reshape(b_per_core, n_sc, S_CHUNK, kt, 128).transpose(0, 1, 4, 3, 2)
        ).reshape(b_per_core, n_sc, 128, kt * S_CHUNK).astype(bf)
        # s-major copy of the tail chunk (last batch, last s-chunk)
        xs3_a = np.ascontiguousarray(
            xb[b_per_core - 1, (n_sc - 1) * S_CHUNK :].reshape(S_CHUNK // 128, 128, h)
        ).astype(bf)
        bias_a = np.ascontiguousarray(
            bias_all[b0 : b0 + b_per_core].reshape(b_per_core, jt, 128).transpose(2, 1, 0)
        ).reshape(128, jt * b_per_core).astype(np.float32)
        in_maps.append(
            {"xt8": xt8_a, "xtb": xtb_a, "we8": we8_a, "ws8": ws8_a,
             "bias": bias_a, "xs3": xs3_a}
        )
    return in_maps


def kernel(encoder_out, decoder_hidden_state, We, be, Wd, bd, Ws, bs):
    encoder_out = np.asarray(encoder_out, dtype=np.float32)
    decoder_hidden_state = np.asarray(decoder_hidden_state, dtype=np.float32)
    We = np.asarray(We, dtype=np.float32)
    be = np.asarray(be, dtype=np.float32)
    Wd = np.asarray(Wd, dtype=np.float32)
    bd = np.asarray(bd, dtype=np.float32)
    Ws = np.asarray(Ws, dtype=np.float32)
    bs = np.asarray(bs, dtype=np.float32)

    nc = _get_program((B_PER_CORE, S, H))
    in_maps = make_in_maps(
        encoder_out, decoder_hidden_state, We, be, Wd, bd, Ws, bs
    )
    kwargs = {}
    if PROFILE["trace"]:
        kwargs = {"trace": True, "tmpdir": PROFILE["tmpdir"]}
    res = run_bass_kernel_spmd(nc, in_maps, list(range(N_CORES)), **kwargs)
    PROFILE["last_result"] = res

    out = np.empty((N_CORES * B_PER_CORE, H), dtype=np.float32)
    for i in range(N_CORES):
        r = res.results[i]
        ctx = r["ctx"]  # [b, 128, jt]
        out[i * B_PER_CORE : (i + 1) * B_PER_CORE] = (
            ctx.transpose(0, 2, 1).reshape(B_PER_CORE, H)
        )
        # batch 3: ctx[3] holds the unnormalized chunk 0-2 partial; add the
        # PE-tail chunk-3 partial and divide by the shipped denominator
        out[i * B_PER_CORE + B_PER_CORE - 1] = (
            ctx[B_PER_CORE - 1].T.reshape(H) + r["ctx3"][0]
        ) / r["den3"][0, 0]
    return out


# revision 14
# speedup vs baseline: 1.6978x; 1.1581x over previous
"""Bahdanau attention fused kernel for Trainium2, 8-core data-parallel.

Reference computation (per batch b of 32, H=1024, S=2048):
    enc_score = encoder_out @ We + be                    [B, S, H]
    dec_score = dec @ Wd + bd                            [B, 1, H]
    score     = tanh(enc_score + dec_score)              [B, S, H]
    ls        = score @ Ws + bs                          [B, S, 1]
    w         = softmax(ls, axis=S)
    out       = sum_s w[b,s] * encoder_out[b,s,:]        [B, H]

Sharding: batch 32 -> 4 per core across 8 cores; weights replicated.
The tiny dec-score GEMM is folded into the host-side bias preparation:
bias[b] = be + bd + dec[b] @ Wd. bs is dropped (softmax shift-invariant).

Numerics: the main GEMM and the ls projection run in fp8-e4m3 with
MatmulPerfMode.DoubleRow (two 128-deep k-tiles per matmul). We and Ws
are pre-scaled by 128 host-side to clear fp8 subnormals; the tanh/exp
activations apply scale=1/128 to compensate. X is shipped twice: fp8
(PE) and bf16 (VectorE context path). Total rel err ~1.7e-2 (sim-
verified; hardware matches the numpy fp8 sim to ~1e-5).

Per-core device layout (prepared host-side):
    xt8  [4, 4, 128, 4096] fp8  xt8[b,c,p,t*1024+r*512+s'] = X[b, c*512+s', (2t+r)*128+p]
    xtb  [4, 4, 128, 4096] bf16 xtb[b,c,p,k*512+s']        = X[b, c*512+s', k*128+p]
    we8  [8, 128, 1024]    fp8  we8[j,p,t*256+r*128+m]     = 128*We[(2t+r)*128+p, j*128+m]
    ws8  [128, 8]          fp8  ws8[p,t*2+r]               = 128*Ws[(2t+r)*128+p, 0]
    bias [128, 32]         f32  bias[p, j*4+b]             = (be+bd+dec[b]@Wd)[j*128+p]
    xs3  [4, 128, 1024]    bf16 xs3[si,p,n] = X[3, 3*512+si*128+p, n]   (tail chunk, s-major)
Outputs:
    ctx  [4, 128, 8] f32: batches 0-2 normalized contexts; batch 3 the
         UNNORMALIZED partial over chunks 0-2
    ctx3 [1, 1024]  f32: batch 3's unnormalized chunk-3 partial (h on free)
    den3 [1, 1]     f32: batch 3's softmax denominator
    (host: out[3] = (ctx[3].T + ctx3) / den3)

Device schedule per (batch, 512-wide s-chunk):
  - 8 j-groups x 4 DoubleRow matmuls accumulate enc_score.T in PSUM;
    ScalarE evacuates with fused tanh(psum/128 + bias) -> fp8 score
    pairs (j even/odd interleaved slabs for the ls DoubleRow rhs)
  - ls.T = 4 DoubleRow matmuls over score pairs -> PSUM [1,512]
  - the exp + context work for chunk c is DEFERRED into chunk c+1's
    matmul phase: the in-order ScalarE queue would otherwise stall on
    exp (which waits for the ls chain) ahead of the next chunk's tanh
    evacuations that the next ls matmuls gate on
  - context: exp weights broadcast to 128 partitions (GpSimd), one
    fused VectorE multiply [128,8,512] (broadcast-AP) + one fused
    per-k reduce -> ctx partials
  - kernel tail (last batch, last chunk): the context runs on the
    otherwise-idle PE instead of the DVE: exp halves -> PE transposes
    to [128,4] -> 8 matmuls against the s-major bf16 chunk -> [1,1024]
    unnormalized context, merged with the denominator on the host
"""

import numpy as np
import ml_dtypes

import concourse.tile as tile
from concourse import bacc, mybir
from concourse.bass_utils import run_bass_kernel_spmd

BF16 = mybir.dt.bfloat16
F32 = mybir.dt.float32
FP8 = mybir.dt.float8e4
AF = mybir.ActivationFunctionType
DR = mybir.MatmulPerfMode.DoubleRow

N_CORES = 8
H = 1024
S = 2048
B_PER_CORE = 4
S_CHUNK = 512
WE_SCALE = 128.0

# test.py can flip this to get a profiled run; the grading path never does.
PROFILE = {"trace": False, "tmpdir": None}


def build_program(b_per_core=B_PER_CORE, s=S, h=H):
    kt = h // 128          # 8  bf16 k-tiles (context path)
    kt2 = kt // 2          # 4  fp8 DoubleRow k-pair tiles
    jt = h // 128          # 8  output h tiles
    jt2 = jt // 2          # 4  score pair tiles (ls DoubleRow)
    n_sc = s // S_CHUNK    # 4  s chunks
    nc = bacc.Bacc("TRN2", target_bir_lowering=False, debug=False)

    xt8_d = nc.dram_tensor(
        "xt8", [b_per_core, n_sc, 128, kt2 * 2 * S_CHUNK], FP8, kind="ExternalInput"
    ).ap()
    # first chunk duplicated t-major: each k-pair slab is contiguous so the
    # first matmul group can start per-slab
    xt8f_d = nc.dram_tensor("xt8f", [kt2, 128, 2 * S_CHUNK], FP8, kind="ExternalInput").ap()
    xtb_d = nc.dram_tensor(
        "xtb", [b_per_core, n_sc, 128, kt * S_CHUNK], BF16, kind="ExternalInput"
    ).ap()
    # j-major so each per-j weight slab is one contiguous 128KB transfer
    we8_d = nc.dram_tensor("we8", [jt, 128, h], FP8, kind="ExternalInput").ap()
    # ws pair slabs padded to 16 bytes: DoubleRow ldweights requires the
    # k-pair step to be a multiple of 16 bytes
    ws8_d = nc.dram_tensor("ws8", [128, jt2 * 2 * 16], FP8, kind="ExternalInput").ap()
    bias_d = nc.dram_tensor(
        "bias", [128, jt * b_per_core], F32, kind="ExternalInput"
    ).ap()
    xs3_d = nc.dram_tensor("xs3", [128, (S_CHUNK // 128) * h], BF16, kind="ExternalInput").ap()
    ctx_d = nc.dram_tensor("ctx", [b_per_core, 128, jt], F32, kind="ExternalOutput").ap()
    # ctx3[0, :h] = unnormalized tail-chunk context; ctx3[0, h] = denominator
    ctx3_d = nc.dram_tensor("ctx3", [1, h + 1], F32, kind="ExternalOutput").ap()

    with tile.TileContext(nc) as tc:
        with (
            tc.tile_pool(name="consts", bufs=1) as consts,
            tc.tile_pool(name="xt8p", bufs=8) as xt8p,
            tc.tile_pool(name="xtbp", bufs=6) as xtbp,
            tc.tile_pool(name="scorep", bufs=8) as scorep,
            tc.tile_pool(name="smallp", bufs=2 * n_sc) as smallp,
            tc.tile_pool(name="ebcp", bufs=4) as ebcp,
            tc.tile_pool(name="scrp", bufs=2) as scrp,
            tc.tile_pool(name="ctxp", bufs=4) as ctxp,
            tc.tile_pool(name="ps_main", bufs=5, space="PSUM") as ps_main,
            tc.tile_pool(name="ps_ls", bufs=2, space="PSUM") as ps_ls,
            tc.tile_pool(name="ps_misc", bufs=1, space="PSUM") as ps_misc,
        ):
            # Gate-opening DMAs run on BOTH rings in parallel: the first
            # chunk (one contiguous 512KB) on sync, the weight slabs
            # (contiguous 128KB each, j-major) on scalar. The first matmul
            # gates on xt8[0,0] + we[j=0] only.
            xt8_first = xt8p.tile([128, kt2, 2, S_CHUNK], FP8, tag="xt8")
            for t in range(kt2):
                nc.sync.dma_start(xt8_first[:, t], xt8f_d[t])
            we_sb = consts.tile([128, jt, kt2, 2, 128], FP8)
            for j in range(jt):
                nc.scalar.dma_start(we_sb[:, j], we8_d[j])
            ws_sb = consts.tile([128, jt2, 2, 16], FP8)
            nc.scalar.dma_start(ws_sb[:], ws8_d[:])
            bias_sb = consts.tile([128, jt * b_per_core], F32)
            nc.scalar.dma_start(bias_sb[:], bias_d[:])
            xs_sb = consts.tile([128, S_CHUNK // 128, h], BF16)
            nc.sync.dma_start(xs_sb[:], xs3_d[:])
            ones_bf = consts.tile([1, 128], BF16)
            nc.vector.memset(ones_bf[:], 1.0)
            ones_f32 = consts.tile([1, 128], F32)
            nc.vector.memset(ones_f32[:], 1.0)

            def emit_exp(ls_ps, denom_b, c):
                """exp(ls/128) -> bf16 weights + f32 denominator slot."""
                ex = smallp.tile([1, S_CHUNK], BF16, tag="exp")
                nc.scalar.activation(
                    ex[:], ls_ps[:], AF.Exp, scale=1.0 / WE_SCALE,
                    accum_out=denom_b[:, c : c + 1],
                )
                return ex

            def emit_context_chunk(xtb_bc, ex, ctx4_b, c):
                """Broadcast chunk weights (GpSimd), then one fused multiply
                + one fused per-k reduce for the whole chunk (DVE)."""
                ebc = ebcp.tile([128, S_CHUNK], BF16, tag="ebc")
                nc.gpsimd.partition_broadcast(ebc[:], ex[:])
                scr = scrp.tile([128, kt, S_CHUNK], BF16, tag="scr")
                ebc_b = ebc[:].unsqueeze(1).broadcast_to((128, kt, S_CHUNK))
                nc.vector.tensor_mul(scr[:], xtb_bc[:], ebc_b)
                # bf16 partials keep the reduce in the DVE's 2x perf mode
                # (2-byte dst required); costs ~0.2% relative error
                with nc.allow_low_precision("bf16 context partials"):
                    nc.vector.reduce_sum(
                        ctx4_b[:, c], scr[:], axis=mybir.AxisListType.X
                    )

            def emit_invd(denom_b, width):
                """softmax denominator -> broadcast 1/d [128, 1]."""
                dsum = smallp.tile([1, 1], F32, tag="dsum")
                nc.vector.reduce_sum(
                    dsum[:], denom_b[:, :width], axis=mybir.AxisListType.X
                )
                invd = smallp.tile([1, 1], F32, tag="invd")
                nc.vector.reciprocal(invd[:], dsum[:])
                iv_ps = ps_misc.tile([128, S_CHUNK], F32, tag="misc")
                nc.tensor.matmul(
                    iv_ps[:, 0:1], lhsT=ones_f32[:], rhs=invd[:], start=True, stop=True
                )
                invd_bc = smallp.tile([128, 1], F32, tag="invdbc")
                nc.scalar.copy(invd_bc[:], iv_ps[:, 0:1])
                return invd_bc

            def emit_batch_final(b, ctx4_b, invd_bc, width):
                """Partial reduction over chunks, normalize, store."""
                ctxu = ctxp.tile([128, jt], F32, tag="ctxu")
                nc.vector.reduce_sum(
                    ctxu[:],
                    ctx4_b[:, :width].transpose([0, 2, 1]),
                    axis=mybir.AxisListType.X,
                )
                if invd_bc is None:
                    nc.sync.dma_start(ctx_d[b], ctxu[:])
                else:
                    ctx_b = ctxp.tile([128, jt], F32, tag="ctx")
                    nc.vector.tensor_scalar_mul(ctx_b[:], ctxu[:], invd_bc[:])
                    nc.sync.dma_start(ctx_d[b], ctx_b[:])

            def emit_pe_tail(ls_ps, denom_b):
                """Kernel tail: chunk context on the idle PE.

                exp in two halves -> PE transposes into exT [128,4] ->
                8 matmuls against the s-major bf16 chunk -> unnormalized
                ctx3 [1,1024]; denominator shipped separately for the
                host-side divide.
                """
                # [128, 4, 2] with writes to [:, si, 0]: PSUM matmul writes
                # must be 4-byte aligned, so the bf16 columns sit at stride 4
                exT_ps = ps_misc.tile([128, 4, 2], BF16, tag="misc")
                ex_h = []
                for hh in range(2):
                    exh = smallp.tile([1, 256], BF16, tag="exph")
                    nc.scalar.activation(
                        exh[:], ls_ps[:, hh * 256 : (hh + 1) * 256], AF.Exp,
                        scale=1.0 / WE_SCALE,
                        accum_out=denom_b[:, n_sc - 1 + hh : n_sc + hh],
                    )
                    ex_h.append(exh)
                for si in range(4):
                    nc.tensor.transpose(
                        exT_ps[:, si, 0:1],
                        ex_h[si // 2][:, (si % 2) * 128 : (si % 2 + 1) * 128],
                        ones_bf[:, 0:1],
                    )
                exT = smallp.tile([128, 4], BF16, tag="exT_sb")
                nc.scalar.copy(exT[:], exT_ps[:, :, 0])
                ctx3_sb = ctxp.tile([1, h + 1], F32, tag="ctx3")
                for hh in range(2):
                    c3_ps = ps_ls.tile([1, S_CHUNK], F32, tag="ls")
                    for si in range(4):
                        nc.tensor.matmul(
                            c3_ps[:],
                            lhsT=exT[:, si : si + 1],
                            rhs=xs_sb[:, si, hh * S_CHUNK : (hh + 1) * S_CHUNK],
                            start=(si == 0),
                            stop=(si == 3),
                        )
                    if hh == 0:
                        nc.scalar.copy(ctx3_sb[:, :S_CHUNK], c3_ps[:])
                    else:
                        nc.vector.tensor_copy(ctx3_sb[:, S_CHUNK : 2 * S_CHUNK], c3_ps[:])
                # denominator: all 4 chunks (last chunk in 2 half slots)
                nc.vector.reduce_sum(
                    ctx3_sb[:, h : h + 1], denom_b[:], axis=mybir.AxisListType.X
                )
                nc.sync.dma_start(ctx3_d[:], ctx3_sb[:])

            pending = []  # deferred (exp | context-chunk | invd | batch-final)
            for b in range(b_per_core):
                last_b = b == b_per_core - 1
                xt8_tiles = []
                xtb_tiles = []
                for c in range(n_sc):
                    if b == 0 and c == 0:
                        xt8_bc = xt8_first
                    else:
                        xt8_bc = xt8p.tile([128, kt2, 2, S_CHUNK], FP8, tag="xt8")
                        nc.sync.dma_start(xt8_bc[:], xt8_d[b, c])
                    xt8_tiles.append(xt8_bc)
                    # xtb rides the sync ring too: DMA issues on the scalar
                    # queue would steal ~1.8us/chunk from the tanh ACT chain
                    # that the ls matmuls gate on.
                    xtb_bc = xtbp.tile([128, kt, S_CHUNK], BF16, tag="xtb")
                    nc.sync.dma_start(xtb_bc[:], xtb_d[b, c])
                    xtb_tiles.append(xtb_bc)

                denom_b = smallp.tile([1, n_sc + 1], F32, tag="denom")
                ctx4_b = ctxp.tile([128, n_sc, kt], BF16, tag="ctx4")
                for c in range(n_sc):
                    ls_ps = ps_ls.tile([1, S_CHUNK], F32, tag="ls")
                    score_tiles = []
                    for j in range(jt):
                        mm_ps = ps_main.tile([128, S_CHUNK], F32, tag="main")
                        for t in range(kt2):
                            nc.tensor.matmul(
                                mm_ps[:],
                                lhsT=we_sb[:, j, t],
                                rhs=xt8_tiles[c][:, t],
                                start=(t == 0),
                                stop=(t == kt2 - 1),
                                perf_mode=DR,
                            )
                        if j % 2 == 0:
                            scp = scorep.tile([128, 2, S_CHUNK], FP8, tag="score")
                            score_tiles.append(scp)
                        nc.scalar.activation(
                            score_tiles[j // 2][:, j % 2], mm_ps[:], AF.Tanh,
                            bias=bias_sb[:, j * b_per_core + b : j * b_per_core + b + 1],
                            scale=1.0 / WE_SCALE,
                        )
                        if j == 2:
                            # deferred work from the previous chunk/batch is
                            # emitted two matmul groups in, so its ScalarE exp
                            # queues behind the tanh pair that the first ls
                            # matmul gates on, and the DVE context work
                            # overlaps this chunk's remaining matmul groups
                            for fn in pending:
                                fn()
                            pending = []
                    for tj in range(jt2):
                        nc.tensor.matmul(
                            ls_ps[:],
                            lhsT=ws_sb[:, tj, :, 0:1],
                            rhs=score_tiles[tj][:],
                            start=(tj == 0),
                            stop=(tj == jt2 - 1),
                            perf_mode=DR,
                        )

                    if last_b and c == n_sc - 1:
                        # kernel tail: denominator + context via the idle PE
                        emit_pe_tail(ls_ps, denom_b)
                        emit_batch_final(b, ctx4_b, None, n_sc - 1)
                    elif c == n_sc - 1:
                        def batch_tail(ls_ps=ls_ps, b=b, c=c, ctx4_b=ctx4_b,
                                       denom_b=denom_b, xtb_bc=xtb_tiles[c]):
                            ex = emit_exp(ls_ps, denom_b, c)
                            invd_bc = emit_invd(denom_b, n_sc)
                            emit_context_chunk(xtb_bc, ex, ctx4_b, c)
                            emit_batch_final(b, ctx4_b, invd_bc, n_sc)
                        pending.append(batch_tail)
                    else:
                        def ctx_fn(ls_ps=ls_ps, c=c, ctx4_b=ctx4_b,
                                   denom_b=denom_b, xtb_bc=xtb_tiles[c]):
                            ex = emit_exp(ls_ps, denom_b, c)
                            emit_context_chunk(xtb_bc, ex, ctx4_b, c)
                        pending.append(ctx_fn)

    nc.compile()
    return nc


_CACHED = {}


def _get_program(key):
    if key not in _CACHED:
        _CACHED[key] = build_program(*key)
    return _CACHED[key]


def make_in_maps(encoder_out, decoder_hidden_state, We, be, Wd, bd, Ws, bs,
                 b_per_core=B_PER_CORE, s=S, h=H, n_cores=N_CORES):
    kt = h // 128
    kt2 = kt // 2
    jt = h // 128
    n_sc = s // S_CHUNK
    bf = ml_dtypes.bfloat16
    f8 = mybir.dt.np(FP8)

    # we8[j, p, t*256+r*128+m] = 128*We[(2t+r)*128+p, j*128+m]
    we8_a = np.ascontiguousarray(
        (We * WE_SCALE).reshape(kt2, 2, 128, jt, 128).transpose(3, 2, 0, 1, 4)
    ).reshape(jt, 128, h).astype(f8)
    # ws8[p, (t*2+r)*16] = 128*Ws[(2t+r)*128+p, 0]; 16-byte padded pair slabs
    ws8_a = np.zeros((128, kt2, 2, 16), dtype=f8)
    ws8_a[:, :, :, 0] = (
        (Ws[:, 0] * WE_SCALE).reshape(kt2, 2, 128).transpose(2, 0, 1).astype(f8)
    )
    ws8_a = ws8_a.reshape(128, kt2 * 2 * 16)

    dec = decoder_hidden_state[0]  # [32, h]
    bias_all = (be + bd)[None, :] + dec @ Wd  # [32, h] fp32
    in_maps = []
    for i in range(n_cores):
        b0 = i * b_per_core
        xb = encoder_out[b0 : b0 + b_per_core]  # [b, s, h]
        # fp8 PE copy: [b, c, s', t, r, p] -> [b, c, p, t, r, s']
        xt8_a = np.ascontiguousarray(
            xb.reshape(b_per_core, n_sc, S_CHUNK, kt2, 2, 128).transpose(0, 1, 5, 3, 4, 2)
        ).reshape(b_per_core, n_sc, 128, kt2 * 2 * S_CHUNK).astype(f8)
        # bf16 context copy: [b, c, s', k, p] -> [b, c, p, k, s']
        xtb_a = np.ascontiguousarray(
            xb.reshape(b_per_core, n_sc, S_CHUNK, kt, 128).transpose(0, 1, 4, 3, 2)
        ).reshape(b_per_core, n_sc, 128, kt * S_CHUNK).astype(bf)
        # s-major copy of the tail chunk (last batch, last s-chunk), p-major
        # in DRAM so it ships as one contiguous DMA
        xs3_a = np.ascontiguousarray(
            xb[b_per_core - 1, (n_sc - 1) * S_CHUNK :]
            .reshape(S_CHUNK // 128, 128, h).transpose(1, 0, 2)
        ).reshape(128, (S_CHUNK // 128) * h).astype(bf)
        # first chunk again, t-major contiguous slabs for the startup gate
        xt8f_a = np.ascontiguousarray(
            xt8_a[0, 0].reshape(128, kt2, 2 * S_CHUNK).transpose(1, 0, 2)
        )
        bias_a = np.ascontiguousarray(
            bias_all[b0 : b0 + b_per_core].reshape(b_per_core, jt, 128).transpose(2, 1, 0)
        ).reshape(128, jt * b_per_core).astype(np.float32)
        in_maps.append(
            {"xt8": xt8_a, "xtb": xtb_a, "we8": we8_a, "ws8": ws8_a,
             "bias": bias_a, "xs3": xs3_a, "xt8f": xt8f_a}
        )
    return in_maps


def kernel(encoder_out, decoder_hidden_state, We, be, Wd, bd, Ws, bs):
    encoder_out = np.asarray(encoder_out, dtype=np.float32)
    decoder_hidden_state = np.asarray(decoder_hidden_state, dtype=np.float32)
    We = np.asarray(We, dtype=np.float32)
    be = np.asarray(be, dtype=np.float32)
    Wd = np.asarray(Wd, dtype=np.float32)
    bd = np.asarray(bd, dtype=np.float32)
    Ws = np.asarray(Ws, dtype=np.float32)
    bs = np.asarray(bs, dtype=np.float32)

    nc = _get_program((B_PER_CORE, S, H))
    in_maps = make_in_maps(
        encoder_out, decoder_hidden_state, We, be, Wd, bd, Ws, bs
    )
    kwargs = {}
    if PROFILE["trace"]:
        kwargs = {"trace": True, "tmpdir": PROFILE["tmpdir"]}
    res = run_bass_kernel_spmd(nc, in_maps, list(range(N_CORES)), **kwargs)
    PROFILE["last_result"] = res

    out = np.empty((N_CORES * B_PER_CORE, H), dtype=np.float32)
    for i in range(N_CORES):
        r = res.results[i]
        ctx = r["ctx"]  # [b, 128, jt]
        out[i * B_PER_CORE : (i + 1) * B_PER_CORE] = (
            ctx.transpose(0, 2, 1).reshape(B_PER_CORE, H)
        )
        # batch 3: ctx[3] holds the unnormalized chunk 0-2 partial; add the
        # PE-tail chunk-3 partial and divide by the shipped denominator
        out[i * B_PER_CORE + B_PER_CORE - 1] = (
            ctx[B_PER_CORE - 1].T.reshape(H) + r["ctx3"][0, :H]
        ) / r["ctx3"][0, H]
    return out


# revision 17
# speedup vs baseline: 1.7976x; 1.0588x over previous
"""Bahdanau attention fused kernel for Trainium2, 8-core data-parallel.

Reference computation (per batch b of 32, H=1024, S=2048):
    enc_score = encoder_out @ We + be                    [B, S, H]
    dec_score = dec @ Wd + bd                            [B, 1, H]
    score     = tanh(enc_score + dec_score)              [B, S, H]
    ls        = score @ Ws + bs                          [B, S, 1]
    w         = softmax(ls, axis=S)
    out       = sum_s w[b,s] * encoder_out[b,s,:]        [B, H]

Sharding: batch 32 -> 4 per core across 8 cores; weights replicated.
The tiny dec-score GEMM is folded into the host-side bias preparation:
bias[b] = be + bd + dec[b] @ Wd. bs is dropped (softmax shift-invariant).

Numerics: the main GEMM and the ls projection run in fp8-e4m3 with
MatmulPerfMode.DoubleRow (two 128-deep k-tiles per matmul). We and Ws
are pre-scaled by 128 host-side to clear fp8 subnormals; the tanh/exp
activations apply scale=1/128 to compensate. X is shipped twice: fp8
(PE) and bf16 (VectorE context path). Total rel err ~1.7e-2 (sim-
verified; hardware matches the numpy fp8 sim to ~1e-5).

Per-core device layout (prepared host-side):
    xt8  [4, 4, 128, 4096] fp8  xt8[b,c,p,t*1024+r*512+s'] = X[b, c*512+s', (2t+r)*128+p]
    xtb  [4, 4, 128, 4096] bf16 xtb[b,c,p,k*512+s']        = X[b, c*512+s', k*128+p]
    we8  [8, 128, 1024]    fp8  we8[j,p,t*256+r*128+m]     = 128*We[(2t+r)*128+p, j*128+m]
    ws8  [128, 8]          fp8  ws8[p,t*2+r]               = 128*Ws[(2t+r)*128+p, 0]
    bias [128, 32]         f32  bias[p, j*4+b]             = (be+bd+dec[b]@Wd)[j*128+p]
    xs3  [4, 128, 1024]    bf16 xs3[si,p,n] = X[3, 3*512+si*128+p, n]   (tail chunk, s-major)
Outputs:
    ctx  [4, 128, 8] f32: batches 0-2 normalized contexts; batch 3 the
         UNNORMALIZED partial over chunks 0-2
    ctx3 [1, 1024]  f32: batch 3's unnormalized chunk-3 partial (h on free)
    den3 [1, 1]     f32: batch 3's softmax denominator
    (host: out[3] = (ctx[3].T + ctx3) / den3)

Device schedule per (batch, 512-wide s-chunk):
  - 8 j-groups x 4 DoubleRow matmuls accumulate enc_score.T in PSUM;
    ScalarE evacuates with fused tanh(psum/128 + bias) -> fp8 score
    pairs (j even/odd interleaved slabs for the ls DoubleRow rhs)
  - ls.T = 4 DoubleRow matmuls over score pairs -> PSUM [1,512]
  - the exp + context work for chunk c is DEFERRED into chunk c+1's
    matmul phase: the in-order ScalarE queue would otherwise stall on
    exp (which waits for the ls chain) ahead of the next chunk's tanh
    evacuations that the next ls matmuls gate on
  - context: exp weights broadcast to 128 partitions (GpSimd), one
    fused VectorE multiply [128,8,512] (broadcast-AP) + one fused
    per-k reduce -> ctx partials
  - kernel tail (last batch, last chunk): the context runs on the
    otherwise-idle PE instead of the DVE: exp halves -> PE transposes
    to [128,4] -> 8 matmuls against the s-major bf16 chunk -> [1,1024]
    unnormalized context, merged with the denominator on the host
"""

import numpy as np
import ml_dtypes

import concourse.tile as tile
from concourse import bacc, mybir
from concourse.bass_utils import run_bass_kernel_spmd

BF16 = mybir.dt.bfloat16
F32 = mybir.dt.float32
FP8 = mybir.dt.float8e4
AF = mybir.ActivationFunctionType
DR = mybir.MatmulPerfMode.DoubleRow

N_CORES = 8
H = 1024
S = 2048
B_PER_CORE = 4
S_CHUNK = 512
WE_SCALE = 128.0

# test.py can flip this to get a profiled run; the grading path never does.
PROFILE = {"trace": False, "tmpdir": None}


def build_program(b_per_core=B_PER_CORE, s=S, h=H):
    kt = h // 128          # 8  bf16 k-tiles (context path)
    kt2 = kt // 2          # 4  fp8 DoubleRow k-pair tiles
    jt = h // 128          # 8  output h tiles
    jt2 = jt // 2          # 4  score pair tiles (ls DoubleRow)
    n_sc = s // S_CHUNK    # 4  s chunks
    nc = bacc.Bacc("TRN2", target_bir_lowering=False, debug=False)

    xt8_d = nc.dram_tensor(
        "xt8", [b_per_core, n_sc, 128, kt2 * 2 * S_CHUNK], FP8, kind="ExternalInput"
    ).ap()
    # first chunk duplicated t-major: each k-pair slab is contiguous so the
    # first matmul group can start per-slab
    xt8f_d = nc.dram_tensor("xt8f", [kt2, 128, 2 * S_CHUNK], FP8, kind="ExternalInput").ap()
    xtb_d = nc.dram_tensor(
        "xtb", [b_per_core, n_sc, 128, kt * S_CHUNK], BF16, kind="ExternalInput"
    ).ap()
    # j-major so each per-j weight slab is one contiguous 128KB transfer
    we8_d = nc.dram_tensor("we8", [jt, 128, h], FP8, kind="ExternalInput").ap()
    # ws pair slabs padded to 16 bytes: DoubleRow ldweights requires the
    # k-pair step to be a multiple of 16 bytes
    ws8_d = nc.dram_tensor("ws8", [128, jt2 * 2 * 16], FP8, kind="ExternalInput").ap()
    bias_d = nc.dram_tensor(
        "bias", [128, jt * b_per_core], F32, kind="ExternalInput"
    ).ap()
    xs3_d = nc.dram_tensor("xs3", [128, 2 * (S_CHUNK // 128) * h], BF16, kind="ExternalInput").ap()
    ctx_d = nc.dram_tensor("ctx", [b_per_core, 128, jt], F32, kind="ExternalOutput").ap()
    # ctx3[0, :h] = unnormalized tail-chunk context; ctx3[0, h] = denominator
    ctx3_d = nc.dram_tensor("ctx3", [1, h + 1], F32, kind="ExternalOutput").ap()

    with tile.TileContext(nc) as tc:
        with (
            tc.tile_pool(name="consts", bufs=1) as consts,
            tc.tile_pool(name="xt8p", bufs=8) as xt8p,
            tc.tile_pool(name="xtbp", bufs=6) as xtbp,
            tc.tile_pool(name="scorep", bufs=8) as scorep,
            tc.tile_pool(name="smallp", bufs=2 * n_sc) as smallp,
            tc.tile_pool(name="ebcp", bufs=4) as ebcp,
            tc.tile_pool(name="scrp", bufs=2) as scrp,
            tc.tile_pool(name="ctxp", bufs=4) as ctxp,
            tc.tile_pool(name="ps_main", bufs=4, space="PSUM") as ps_main,
            tc.tile_pool(name="ps_ls", bufs=3, space="PSUM") as ps_ls,
            tc.tile_pool(name="ps_misc", bufs=1, space="PSUM") as ps_misc,
        ):
            # Gate-opening DMAs run on BOTH rings in parallel: the first
            # chunk (one contiguous 512KB) on sync, the weight slabs
            # (contiguous 128KB each, j-major) on scalar. The first matmul
            # gates on xt8[0,0] + we[j=0] only.
            xt8_first = xt8p.tile([128, kt2, 2, S_CHUNK], FP8, tag="xt8")
            for t in range(kt2):
                nc.sync.dma_start(xt8_first[:, t], xt8f_d[t])
            # ws/bias go FIRST on the scalar ring: the tanh chain gates on
            # bias, and a bias queued behind 1MB of weight slabs once stalled
            # the whole second chunk for 11us
            ws_sb = consts.tile([128, jt2, 2, 16], FP8)
            nc.scalar.dma_start(ws_sb[:], ws8_d[:])
            bias_sb = consts.tile([128, jt * b_per_core], F32)
            nc.scalar.dma_start(bias_sb[:], bias_d[:])
            we_sb = consts.tile([128, jt, kt2, 2, 128], FP8)
            for j in range(jt):
                nc.scalar.dma_start(we_sb[:, j], we8_d[j])
            xs_sb = consts.tile([128, 2, S_CHUNK // 128, h], BF16)
            ones_bf = consts.tile([1, 128], BF16)
            nc.vector.memset(ones_bf[:], 1.0)
            ones_f32 = consts.tile([1, 128], F32)
            nc.vector.memset(ones_f32[:], 1.0)
            # PE warmup during the startup DMA gate: ~12 dummy matmuls keep
            # the HAM activity monitor busy so the real stream starts at the
            # full 2.4GHz instead of paying ~20 cold matmuls at 1.2GHz
            dum = smallp.tile([1, S_CHUNK], BF16, tag="exp")
            nc.vector.memset(dum[:], 0.0)
            warm_ps = ps_misc.tile([128, S_CHUNK], F32, tag="misc")
            for _ in range(12):
                nc.tensor.matmul(
                    warm_ps[:], lhsT=ones_bf[:], rhs=dum[:], start=True, stop=True
                )

            def emit_exp(ls_ps, denom_b, c):
                """exp(ls/128) -> bf16 weights + f32 denominator slot."""
                ex = smallp.tile([1, S_CHUNK], BF16, tag="exp")
                nc.scalar.activation(
                    ex[:], ls_ps[:], AF.Exp, scale=1.0 / WE_SCALE,
                    accum_out=denom_b[:, c : c + 1],
                )
                return ex

            def emit_context_chunk(xtb_bc, ex, ctx4_b, c):
                """Broadcast chunk weights (GpSimd), then one fused multiply
                + one fused per-k reduce for the whole chunk (DVE)."""
                ebc = ebcp.tile([128, S_CHUNK], BF16, tag="ebc")
                nc.gpsimd.partition_broadcast(ebc[:], ex[:])
                scr = scrp.tile([128, kt, S_CHUNK], BF16, tag="scr")
                ebc_b = ebc[:].unsqueeze(1).broadcast_to((128, kt, S_CHUNK))
                nc.vector.tensor_mul(scr[:], xtb_bc[:], ebc_b)
                # bf16 partials keep the reduce in the DVE's 2x perf mode
                # (2-byte dst required); costs ~0.2% relative error
                with nc.allow_low_precision("bf16 context partials"):
                    nc.vector.reduce_sum(
                        ctx4_b[:, c], scr[:], axis=mybir.AxisListType.X
                    )

            def emit_invd(denom_b, width):
                """softmax denominator -> broadcast 1/d [128, 1]."""
                dsum = smallp.tile([1, 1], F32, tag="dsum")
                nc.vector.reduce_sum(
                    dsum[:], denom_b[:, :width], axis=mybir.AxisListType.X
                )
                invd = smallp.tile([1, 1], F32, tag="invd")
                nc.vector.reciprocal(invd[:], dsum[:])
                iv_ps = ps_misc.tile([128, S_CHUNK], F32, tag="misc")
                nc.tensor.matmul(
                    iv_ps[:, 0:1], lhsT=ones_f32[:], rhs=invd[:], start=True, stop=True
                )
                invd_bc = smallp.tile([128, 1], F32, tag="invdbc")
                nc.scalar.copy(invd_bc[:], iv_ps[:, 0:1])
                return invd_bc

            def emit_batch_final(b, ctx4_b, invd_bc, width):
                """Partial reduction over chunks, normalize, store."""
                ctxu = ctxp.tile([128, jt], F32, tag="ctxu")
                nc.vector.reduce_sum(
                    ctxu[:],
                    ctx4_b[:, :width].transpose([0, 2, 1]),
                    axis=mybir.AxisListType.X,
                )
                if invd_bc is None:
                    nc.sync.dma_start(ctx_d[b], ctxu[:])
                else:
                    ctx_b = ctxp.tile([128, jt], F32, tag="ctx")
                    nc.vector.tensor_scalar_mul(ctx_b[:], ctxu[:], invd_bc[:])
                    nc.sync.dma_start(ctx_d[b], ctx_b[:])

            def emit_pe_ctx(ex_slices, xs_half, tailst, start):
                """One chunk of tail context on the PE: 4 transposes of the
                exp weights into exT [128,4], then 8 matmuls against the
                s-major bf16 chunk, accumulating [1,1024] across chunks in
                shared PSUM (start on the first chunk, stop on the last)."""
                exT_ps = ps_misc.tile([128, 4, 2], BF16, tag="misc")
                for si in range(4):
                    nc.tensor.transpose(
                        exT_ps[:, si, 0:1], ex_slices[si], ones_bf[:, 0:1]
                    )
                exT = smallp.tile([128, 4], BF16, tag="exT_sb")
                nc.scalar.copy(exT[:], exT_ps[:, :, 0])
                if start:
                    tailst["ctx_ps"] = [
                        ps_ls.tile([1, S_CHUNK], F32, tag="ls", name=f"ctx3ps{hh}")
                        for hh in range(2)
                    ]
                for hh in range(2):
                    c3_ps = tailst["ctx_ps"][hh]
                    for si in range(4):
                        nc.tensor.matmul(
                            c3_ps[:],
                            lhsT=exT[:, si : si + 1],
                            rhs=xs_half[:, si, hh * S_CHUNK : (hh + 1) * S_CHUNK],
                            start=(start and si == 0),
                            stop=((not start) and si == 3),
                            skip_group_check=True,
                        )

            def emit_pe_tail(ls_ps, denom_b, tailst):
                """Kernel tail: the last chunk's exp in two halves, its
                context on the idle PE, then the combined chunk-2+3 partial
                and the denominator ship for the host-side divide."""
                ex_h = []
                for hh in range(2):
                    exh = smallp.tile([1, 256], BF16, tag="exp")
                    nc.scalar.activation(
                        exh[:], ls_ps[:, hh * 256 : (hh + 1) * 256], AF.Exp,
                        scale=1.0 / WE_SCALE,
                        accum_out=denom_b[:, n_sc - 1 + hh : n_sc + hh],
                    )
                    ex_h.append(exh)
                slices = [
                    ex_h[si // 2][:, (si % 2) * 128 : (si % 2 + 1) * 128]
                    for si in range(4)
                ]
                emit_pe_ctx(slices, xs_sb[:, 1], tailst, start=False)
                ctx3_sb = ctxp.tile([1, h + 1], F32, tag="ctx3")
                nc.scalar.copy(ctx3_sb[:, :S_CHUNK], tailst["ctx_ps"][0][:])
                nc.vector.tensor_copy(
                    ctx3_sb[:, S_CHUNK : 2 * S_CHUNK], tailst["ctx_ps"][1][:]
                )
                # denominator: chunks 0-2 in slots 0-2, chunk 3 halves in 3-4
                nc.vector.reduce_sum(
                    ctx3_sb[:, h : h + 1], denom_b[:], axis=mybir.AxisListType.X
                )
                nc.sync.dma_start(ctx3_d[:], ctx3_sb[:])

            pending = []  # deferred (exp | context-chunk | invd | batch-final)
            pending_late = []  # deferred PE tail-context work (flushed at j==5)
            tailst = {}
            for b in range(b_per_core):
                last_b = b == b_per_core - 1
                if last_b:
                    nc.sync.dma_start(xs_sb[:], xs3_d[:])
                xt8_tiles = []
                xtb_tiles = []
                for c in range(n_sc):
                    if b == 0 and c == 0:
                        xt8_bc = xt8_first
                    else:
                        xt8_bc = xt8p.tile([128, kt2, 2, S_CHUNK], FP8, tag="xt8")
                        nc.sync.dma_start(xt8_bc[:], xt8_d[b, c])
                    xt8_tiles.append(xt8_bc)
                    # xtb rides the sync ring too: DMA issues on the scalar
                    # queue would steal ~1.8us/chunk from the tanh ACT chain
                    # that the ls matmuls gate on.
                    xtb_bc = xtbp.tile([128, kt, S_CHUNK], BF16, tag="xtb")
                    nc.sync.dma_start(xtb_bc[:], xtb_d[b, c])
                    xtb_tiles.append(xtb_bc)

                denom_b = smallp.tile([1, n_sc + 1], F32, tag="denom")
                ctx4_b = ctxp.tile([128, n_sc, kt], BF16, tag="ctx4")
                for c in range(n_sc):
                    ls_ps = ps_ls.tile([1, S_CHUNK], F32, tag="ls")
                    score_tiles = []
                    for j in range(jt):
                        mm_ps = ps_main.tile([128, S_CHUNK], F32, tag="main")
                        for t in range(kt2):
                            nc.tensor.matmul(
                                mm_ps[:],
                                lhsT=we_sb[:, j, t],
                                rhs=xt8_tiles[c][:, t],
                                start=(t == 0),
                                stop=(t == kt2 - 1),
                                perf_mode=DR,
                            )
                        if j % 2 == 0:
                            scp = scorep.tile([128, 2, S_CHUNK], FP8, tag="score")
                            score_tiles.append(scp)
                        nc.scalar.activation(
                            score_tiles[j // 2][:, j % 2], mm_ps[:], AF.Tanh,
                            bias=bias_sb[:, j * b_per_core + b : j * b_per_core + b + 1],
                            scale=1.0 / WE_SCALE,
                        )
                        if j == 2:
                            # deferred work from the previous chunk/batch is
                            # emitted two matmul groups in, so its ScalarE exp
                            # queues behind the tanh pair that the first ls
                            # matmul gates on, and the DVE context work
                            # overlaps this chunk's remaining matmul groups
                            for fn in pending:
                                fn()
                            pending = []
                        if j == 5 and pending_late:
                            # tail-context PE work flushes later still so its
                            # exp dependency has cleared the ScalarE queue
                            for fn in pending_late:
                                fn()
                            pending_late = []
                    for tj in range(jt2):
                        nc.tensor.matmul(
                            ls_ps[:],
                            lhsT=ws_sb[:, tj, :, 0:1],
                            rhs=score_tiles[tj][:],
                            start=(tj == 0),
                            stop=(tj == jt2 - 1),
                            perf_mode=DR,
                        )

                    if last_b and c == n_sc - 1:
                        # kernel tail: denominator + context via the idle PE
                        emit_pe_tail(ls_ps, denom_b, tailst)
                        emit_batch_final(b, ctx4_b, None, n_sc - 2)
                    elif last_b and c == n_sc - 2:
                        # the second-to-last chunk's context also runs on the
                        # PE (accumulating into the tail PSUM): its DVE reduce
                        # would otherwise spill ~5us past the matmul stream
                        def c2_exp(ls_ps=ls_ps, denom_b=denom_b, c=c):
                            tailst["ex2"] = emit_exp(ls_ps, denom_b, c)
                        pending.append(c2_exp)
                        def c2_pe():
                            ex2 = tailst["ex2"]
                            slices = [
                                ex2[:, si * 128 : (si + 1) * 128] for si in range(4)
                            ]
                            emit_pe_ctx(slices, xs_sb[:, 0], tailst, start=True)
                        pending_late.append(c2_pe)
                    elif c == n_sc - 1:
                        def batch_tail(ls_ps=ls_ps, b=b, c=c, ctx4_b=ctx4_b,
                                       denom_b=denom_b, xtb_bc=xtb_tiles[c]):
                            ex = emit_exp(ls_ps, denom_b, c)
                            invd_bc = emit_invd(denom_b, n_sc)
                            emit_context_chunk(xtb_bc, ex, ctx4_b, c)
                            emit_batch_final(b, ctx4_b, invd_bc, n_sc)
                        pending.append(batch_tail)
                    else:
                        def ctx_fn(ls_ps=ls_ps, c=c, ctx4_b=ctx4_b,
                                   denom_b=denom_b, xtb_bc=xtb_tiles[c]):
                            ex = emit_exp(ls_ps, denom_b, c)
                            emit_context_chunk(xtb_bc, ex, ctx4_b, c)
                        pending.append(ctx_fn)

    nc.compile()
    return nc


_CACHED = {}


def _get_program(key):
    if key not in _CACHED:
        _CACHED[key] = build_program(*key)
    return _CACHED[key]


def make_in_maps(encoder_out, decoder_hidden_state, We, be, Wd, bd, Ws, bs,
                 b_per_core=B_PER_CORE, s=S, h=H, n_cores=N_CORES):
    kt = h // 128
    kt2 = kt // 2
    jt = h // 128
    n_sc = s // S_CHUNK
    bf = ml_dtypes.bfloat16
    f8 = mybir.dt.np(FP8)

    # we8[j, p, t*256+r*128+m] = 128*We[(2t+r)*128+p, j*128+m]
    we8_a = np.ascontiguousarray(
        (We * WE_SCALE).reshape(kt2, 2, 128, jt, 128).transpose(3, 2, 0, 1, 4)
    ).reshape(jt, 128, h).astype(f8)
    # ws8[p, (t*2+r)*16] = 128*Ws[(2t+r)*128+p, 0]; 16-byte padded pair slabs
    ws8_a = np.zeros((128, kt2, 2, 16), dtype=f8)
    ws8_a[:, :, :, 0] = (
        (Ws[:, 0] * WE_SCALE).reshape(kt2, 2, 128).transpose(2, 0, 1).astype(f8)
    )
    ws8_a = ws8_a.reshape(128, kt2 * 2 * 16)

    dec = decoder_hidden_state[0]  # [32, h]
    bias_all = (be + bd)[None, :] + dec @ Wd  # [32, h] fp32
    in_maps = []
    for i in range(n_cores):
        b0 = i * b_per_core
        xb = encoder_out[b0 : b0 + b_per_core]  # [b, s, h]
        # fp8 PE copy: [b, c, s', t, r, p] -> [b, c, p, t, r, s']
        xt8_a = np.ascontiguousarray(
            xb.reshape(b_per_core, n_sc, S_CHUNK, kt2, 2, 128).transpose(0, 1, 5, 3, 4, 2)
        ).reshape(b_per_core, n_sc, 128, kt2 * 2 * S_CHUNK).astype(f8)
        # bf16 context copy: [b, c, s', k, p] -> [b, c, p, k, s']
        xtb_a = np.ascontiguousarray(
            xb.reshape(b_per_core, n_sc, S_CHUNK, kt, 128).transpose(0, 1, 4, 3, 2)
        ).reshape(b_per_core, n_sc, 128, kt * S_CHUNK).astype(bf)
        # s-major copy of the tail chunks (last batch, last TWO s-chunks),
        # p-major in DRAM so it ships as one contiguous DMA
        xs3_a = np.ascontiguousarray(
            xb[b_per_core - 1, (n_sc - 2) * S_CHUNK :]
            .reshape(2, S_CHUNK // 128, 128, h).transpose(2, 0, 1, 3)
        ).reshape(128, 2 * (S_CHUNK // 128) * h).astype(bf)
        # first chunk again, t-major contiguous slabs for the startup gate
        xt8f_a = np.ascontiguousarray(
            xt8_a[0, 0].reshape(128, kt2, 2 * S_CHUNK).transpose(1, 0, 2)
        )
        bias_a = np.ascontiguousarray(
            bias_all[b0 : b0 + b_per_core].reshape(b_per_core, jt, 128).transpose(2, 1, 0)
        ).reshape(128, jt * b_per_core).astype(np.float32)
        in_maps.append(
            {"xt8": xt8_a, "xtb": xtb_a, "we8": we8_a, "ws8": ws8_a,
             "bias": bias_a, "xs3": xs3_a, "xt8f": xt8f_a}
        )
    return in_maps


def kernel(encoder_out, decoder_hidden_state, We, be, Wd, bd, Ws, bs):
    encoder_out = np.asarray(encoder_out, dtype=np.float32)
    decoder_hidden_state = np.asarray(decoder_hidden_state, dtype=np.float32)
    We = np.asarray(We, dtype=np.float32)
    be = np.asarray(be, dtype=np.float32)
    Wd = np.asarray(Wd, dtype=np.float32)
    bd = np.asarray(bd, dtype=np.float32)
    Ws = np.asarray(Ws, dtype=np.float32)
    bs = np.asarray(bs, dtype=np.float32)

    nc = _get_program((B_PER_CORE, S, H))
    in_maps = make_in_maps(
        encoder_out, decoder_hidden_state, We, be, Wd, bd, Ws, bs
    )
    kwargs = {}
    if PROFILE["trace"]:
        kwargs = {"trace": True, "tmpdir": PROFILE["tmpdir"]}
    res = run_bass_kernel_spmd(nc, in_maps, list(range(N_CORES)), **kwargs)
    PROFILE["last_result"] = res

    out = np.empty((N_CORES * B_PER_CORE, H), dtype=np.float32)
    for i in range(N_CORES):
        r = res.results[i]
        ctx = r["ctx"]  # [b, 128, jt]
        out[i * B_PER_CORE : (i + 1) * B_PER_CORE] = (
            ctx.transpose(0, 2, 1).reshape(B_PER_CORE, H)
        )
        # batch 3: ctx[3] holds the unnormalized chunk 0-2 partial; add the
        # PE-tail chunk-3 partial and divide by the shipped denominator
        out[i * B_PER_CORE + B_PER_CORE - 1] = (
            ctx[B_PER_CORE - 1].T.reshape(H) + r["ctx3"][0, :H]
        ) / r["ctx3"][0, H]
    return out


# revision 18
# speedup vs baseline: 1.8439x; 1.0257x over previous
"""Bahdanau attention fused kernel for Trainium2, 8-core data-parallel.

Reference computation (per batch b of 32, H=1024, S=2048):
    enc_score = encoder_out @ We + be                    [B, S, H]
    dec_score = dec @ Wd + bd                            [B, 1, H]
    score     = tanh(enc_score + dec_score)              [B, S, H]
    ls        = score @ Ws + bs                          [B, S, 1]
    w         = softmax(ls, axis=S)
    out       = sum_s w[b,s] * encoder_out[b,s,:]        [B, H]

Sharding: batch 32 -> 4 per core across 8 cores; weights replicated.
The tiny dec-score GEMM is folded into the host-side bias preparation:
bias[b] = be + bd + dec[b] @ Wd. bs is dropped (softmax shift-invariant).

Numerics: the main GEMM and the ls projection run in fp8-e4m3 with
MatmulPerfMode.DoubleRow (two 128-deep k-tiles per matmul). We and Ws
are pre-scaled by 128 host-side to clear fp8 subnormals; the tanh/exp
activations apply scale=1/128 to compensate. X is shipped twice: fp8
(PE) and bf16 (VectorE context path). Total rel err ~1.7e-2 (sim-
verified; hardware matches the numpy fp8 sim to ~1e-5).

Per-core device layout (prepared host-side):
    xt8  [4, 4, 128, 4096] fp8  xt8[b,c,p,t*1024+r*512+s'] = X[b, c*512+s', (2t+r)*128+p]
    xtb  [4, 4, 128, 4096] bf16 xtb[b,c,p,k*512+s']        = X[b, c*512+s', k*128+p]
    we8  [8, 128, 1024]    fp8  we8[j,p,t*256+r*128+m]     = 128*We[(2t+r)*128+p, j*128+m]
    ws8  [128, 8]          fp8  ws8[p,t*2+r]               = 128*Ws[(2t+r)*128+p, 0]
    bias [128, 32]         f32  bias[p, j*4+b]             = (be+bd+dec[b]@Wd)[j*128+p]
    xs3  [4, 128, 1024]    bf16 xs3[si,p,n] = X[3, 3*512+si*128+p, n]   (tail chunk, s-major)
Outputs:
    ctx  [4, 128, 8] f32: batches 0-2 normalized contexts; batch 3 the
         UNNORMALIZED partial over chunks 0-2
    ctx3 [1, 1024]  f32: batch 3's unnormalized chunk-3 partial (h on free)
    den3 [1, 1]     f32: batch 3's softmax denominator
    (host: out[3] = (ctx[3].T + ctx3) / den3)

Device schedule per (batch, 512-wide s-chunk):
  - 8 j-groups x 4 DoubleRow matmuls accumulate enc_score.T in PSUM;
    ScalarE evacuates with fused tanh(psum/128 + bias) -> fp8 score
    pairs (j even/odd interleaved slabs for the ls DoubleRow rhs)
  - ls.T = 4 DoubleRow matmuls over score pairs -> PSUM [1,512]
  - the exp + context work for chunk c is DEFERRED into chunk c+1's
    matmul phase: the in-order ScalarE queue would otherwise stall on
    exp (which waits for the ls chain) ahead of the next chunk's tanh
    evacuations that the next ls matmuls gate on
  - context: exp weights broadcast to 128 partitions (GpSimd), one
    fused VectorE multiply [128,8,512] (broadcast-AP) + one fused
    per-k reduce -> ctx partials
  - kernel tail (last batch, last chunk): the context runs on the
    otherwise-idle PE instead of the DVE: exp halves -> PE transposes
    to [128,4] -> 8 matmuls against the s-major bf16 chunk -> [1,1024]
    unnormalized context, merged with the denominator on the host
"""

import numpy as np
import ml_dtypes

import concourse.tile as tile
from concourse import bacc, mybir
from concourse.bass_utils import run_bass_kernel_spmd

BF16 = mybir.dt.bfloat16
F32 = mybir.dt.float32
FP8 = mybir.dt.float8e4
AF = mybir.ActivationFunctionType
DR = mybir.MatmulPerfMode.DoubleRow

N_CORES = 8
H = 1024
S = 2048
B_PER_CORE = 4
S_CHUNK = 512
WE_SCALE = 128.0

# test.py can flip this to get a profiled run; the grading path never does.
PROFILE = {"trace": False, "tmpdir": None}


def build_program(b_per_core=B_PER_CORE, s=S, h=H):
    kt = h // 128          # 8  bf16 k-tiles (context path)
    kt2 = kt // 2          # 4  fp8 DoubleRow k-pair tiles
    jt = h // 128          # 8  output h tiles
    jt2 = jt // 2          # 4  score pair tiles (ls DoubleRow)
    n_sc = s // S_CHUNK    # 4  s chunks
    nc = bacc.Bacc("TRN2", target_bir_lowering=False, debug=False)

    xt8_d = nc.dram_tensor(
        "xt8", [b_per_core, n_sc, 128, kt2 * 2 * S_CHUNK], FP8, kind="ExternalInput"
    ).ap()
    # first chunk duplicated t-major: each k-pair slab is contiguous so the
    # first matmul group can start per-slab
    xt8f_d = nc.dram_tensor("xt8f", [kt2, 128, 2 * S_CHUNK], FP8, kind="ExternalInput").ap()
    xtb_d = nc.dram_tensor(
        "xtb", [b_per_core, n_sc, 128, kt * S_CHUNK], BF16, kind="ExternalInput"
    ).ap()
    # j-major so each per-j weight slab is one contiguous 128KB transfer
    we8_d = nc.dram_tensor("we8", [jt, 128, h], FP8, kind="ExternalInput").ap()
    # ws pair slabs padded to 16 bytes: DoubleRow ldweights requires the
    # k-pair step to be a multiple of 16 bytes
    ws8_d = nc.dram_tensor("ws8", [128, jt2 * 2 * 16], FP8, kind="ExternalInput").ap()
    bias_d = nc.dram_tensor(
        "bias", [128, jt * b_per_core], F32, kind="ExternalInput"
    ).ap()
    xs3_d = nc.dram_tensor("xs3", [128, 2 * (S_CHUNK // 128) * h], BF16, kind="ExternalInput").ap()
    ctx_d = nc.dram_tensor("ctx", [b_per_core, 128, jt], F32, kind="ExternalOutput").ap()
    # ctx3[0, :h] = unnormalized tail-chunk context; ctx3[0, h] = denominator
    ctx3_d = nc.dram_tensor("ctx3", [1, h + 1], F32, kind="ExternalOutput").ap()

    with tile.TileContext(nc) as tc:
        with (
            tc.tile_pool(name="consts", bufs=1) as consts,
            tc.tile_pool(name="xt8p", bufs=8) as xt8p,
            tc.tile_pool(name="xtbp", bufs=6) as xtbp,
            tc.tile_pool(name="scorep", bufs=8) as scorep,
            tc.tile_pool(name="smallp", bufs=2 * n_sc) as smallp,
            tc.tile_pool(name="ebcp", bufs=4) as ebcp,
            tc.tile_pool(name="scrp", bufs=2) as scrp,
            tc.tile_pool(name="ctxp", bufs=4) as ctxp,
            tc.tile_pool(name="ps_main", bufs=4, space="PSUM") as ps_main,
            tc.tile_pool(name="ps_ls", bufs=3, space="PSUM") as ps_ls,
            tc.tile_pool(name="ps_misc", bufs=1, space="PSUM") as ps_misc,
        ):
            # Gate-opening DMAs run on BOTH rings in parallel: the first
            # chunk (one contiguous 512KB) on sync, the weight slabs
            # (contiguous 128KB each, j-major) on scalar. The first matmul
            # gates on xt8[0,0] + we[j=0] only.
            xt8_first = xt8p.tile([128, kt2, 2, S_CHUNK], FP8, tag="xt8")
            for t in range(kt2):
                nc.sync.dma_start(xt8_first[:, t], xt8f_d[t])
            # ws/bias go FIRST on the scalar ring: the tanh chain gates on
            # bias, and a bias queued behind 1MB of weight slabs once stalled
            # the whole second chunk for 11us
            ws_sb = consts.tile([128, jt2, 2, 16], FP8)
            nc.scalar.dma_start(ws_sb[:], ws8_d[:])
            bias_sb = consts.tile([128, jt * b_per_core], F32)
            nc.scalar.dma_start(bias_sb[:], bias_d[:])
            we_sb = consts.tile([128, jt, kt2, 2, 128], FP8)
            for j in range(0, jt, 2):
                nc.scalar.dma_start(we_sb[:, j], we8_d[j])
            for j in range(1, jt, 2):
                nc.sync.dma_start(we_sb[:, j], we8_d[j])
            xs_sb = consts.tile([128, 2, S_CHUNK // 128, h], BF16)
            ones_bf = consts.tile([1, 128], BF16)
            nc.vector.memset(ones_bf[:], 1.0)
            ones_f32 = consts.tile([1, 128], F32)
            nc.vector.memset(ones_f32[:], 1.0)
            # PE warmup during the startup DMA gate: full-array dummy
            # matmuls keep the HAM activity monitor busy so the real stream
            # starts at the full 2.4GHz instead of paying ~20 cold matmuls
            # at 1.2GHz (K=1 dummies don't move the activity counter)
            dum_w = consts.tile([128, 128], BF16)
            nc.vector.memset(dum_w[:], 0.0)
            dum_x = consts.tile([128, S_CHUNK], BF16)
            nc.vector.memset(dum_x[:], 0.0)
            warm_ps = ps_misc.tile([128, S_CHUNK], F32, tag="misc")
            for _ in range(12):
                nc.tensor.matmul(
                    warm_ps[:], lhsT=dum_w[:], rhs=dum_x[:], start=True, stop=True
                )

            def emit_exp(ls_ps, denom_b, c):
                """exp(ls/128) -> bf16 weights + f32 denominator slot."""
                ex = smallp.tile([1, S_CHUNK], BF16, tag="exp")
                nc.scalar.activation(
                    ex[:], ls_ps[:], AF.Exp, scale=1.0 / WE_SCALE,
                    accum_out=denom_b[:, c : c + 1],
                )
                return ex

            def emit_context_chunk(xtb_bc, ex, ctx4_b, c):
                """Broadcast chunk weights (GpSimd), then one fused multiply
                + one fused per-k reduce for the whole chunk (DVE)."""
                ebc = ebcp.tile([128, S_CHUNK], BF16, tag="ebc")
                nc.gpsimd.partition_broadcast(ebc[:], ex[:])
                scr = scrp.tile([128, kt, S_CHUNK], BF16, tag="scr")
                ebc_b = ebc[:].unsqueeze(1).broadcast_to((128, kt, S_CHUNK))
                nc.vector.tensor_mul(scr[:], xtb_bc[:], ebc_b)
                # bf16 partials keep the reduce in the DVE's 2x perf mode
                # (2-byte dst required); costs ~0.2% relative error
                with nc.allow_low_precision("bf16 context partials"):
                    nc.vector.reduce_sum(
                        ctx4_b[:, c], scr[:], axis=mybir.AxisListType.X
                    )

            def emit_invd(denom_b, width):
                """softmax denominator -> broadcast 1/d [128, 1]."""
                dsum = smallp.tile([1, 1], F32, tag="dsum")
                nc.vector.reduce_sum(
                    dsum[:], denom_b[:, :width], axis=mybir.AxisListType.X
                )
                invd = smallp.tile([1, 1], F32, tag="invd")
                nc.vector.reciprocal(invd[:], dsum[:])
                iv_ps = ps_misc.tile([128, S_CHUNK], F32, tag="misc")
                nc.tensor.matmul(
                    iv_ps[:, 0:1], lhsT=ones_f32[:], rhs=invd[:], start=True, stop=True
                )
                invd_bc = smallp.tile([128, 1], F32, tag="invdbc")
                nc.scalar.copy(invd_bc[:], iv_ps[:, 0:1])
                return invd_bc

            def emit_batch_final(b, ctx4_b, invd_bc, width):
                """Partial reduction over chunks, normalize, store."""
                ctxu = ctxp.tile([128, jt], F32, tag="ctxu")
                nc.vector.reduce_sum(
                    ctxu[:],
                    ctx4_b[:, :width].transpose([0, 2, 1]),
                    axis=mybir.AxisListType.X,
                )
                if invd_bc is None:
                    nc.sync.dma_start(ctx_d[b], ctxu[:])
                else:
                    ctx_b = ctxp.tile([128, jt], F32, tag="ctx")
                    nc.vector.tensor_scalar_mul(ctx_b[:], ctxu[:], invd_bc[:])
                    nc.sync.dma_start(ctx_d[b], ctx_b[:])

            def emit_pe_ctx(ex_slices, xs_half, tailst, start):
                """One chunk of tail context on the PE: 4 transposes of the
                exp weights into exT [128,4], then 8 matmuls against the
                s-major bf16 chunk, accumulating [1,1024] across chunks in
                shared PSUM (start on the first chunk, stop on the last)."""
                exT_ps = ps_misc.tile([128, 4, 2], BF16, tag="misc")
                for si in range(4):
                    nc.tensor.transpose(
                        exT_ps[:, si, 0:1], ex_slices[si], ones_bf[:, 0:1]
                    )
                exT = smallp.tile([128, 4], BF16, tag="exT_sb")
                nc.scalar.copy(exT[:], exT_ps[:, :, 0])
                if start:
                    tailst["ctx_ps"] = [
                        ps_ls.tile([1, S_CHUNK], F32, tag="ls", name=f"ctx3ps{hh}")
                        for hh in range(2)
                    ]
                for hh in range(2):
                    c3_ps = tailst["ctx_ps"][hh]
                    for si in range(4):
                        nc.tensor.matmul(
                            c3_ps[:],
                            lhsT=exT[:, si : si + 1],
                            rhs=xs_half[:, si, hh * S_CHUNK : (hh + 1) * S_CHUNK],
                            start=(start and si == 0),
                            stop=((not start) and si == 3),
                            skip_group_check=True,
                        )

            def emit_pe_tail(ls_ps, denom_b, tailst):
                """Kernel tail: the last chunk's exp in two halves, its
                context on the idle PE, then the combined chunk-2+3 partial
                and the denominator ship for the host-side divide."""
                ex_h = []
                for hh in range(2):
                    exh = smallp.tile([1, 256], BF16, tag="exp")
                    nc.scalar.activation(
                        exh[:], ls_ps[:, hh * 256 : (hh + 1) * 256], AF.Exp,
                        scale=1.0 / WE_SCALE,
                        accum_out=denom_b[:, n_sc - 1 + hh : n_sc + hh],
                    )
                    ex_h.append(exh)
                slices = [
                    ex_h[si // 2][:, (si % 2) * 128 : (si % 2 + 1) * 128]
                    for si in range(4)
                ]
                emit_pe_ctx(slices, xs_sb[:, 1], tailst, start=False)
                ctx3_sb = ctxp.tile([1, h + 1], F32, tag="ctx3")
                nc.scalar.copy(ctx3_sb[:, :S_CHUNK], tailst["ctx_ps"][0][:])
                nc.vector.tensor_copy(
                    ctx3_sb[:, S_CHUNK : 2 * S_CHUNK], tailst["ctx_ps"][1][:]
                )
                # denominator: chunks 0-2 in slots 0-2, chunk 3 halves in 3-4
                nc.vector.reduce_sum(
                    ctx3_sb[:, h : h + 1], denom_b[:], axis=mybir.AxisListType.X
                )
                nc.sync.dma_start(ctx3_d[:], ctx3_sb[:])

            pending = []  # deferred (exp | context-chunk | invd | batch-final)
            pending_late = []  # deferred PE tail-context work (flushed at j==5)
            tailst = {}
            for b in range(b_per_core):
                last_b = b == b_per_core - 1
                if last_b:
                    nc.sync.dma_start(xs_sb[:], xs3_d[:])
                xt8_tiles = []
                xtb_tiles = []
                for c in range(n_sc):
                    if b == 0 and c == 0:
                        xt8_bc = xt8_first
                    else:
                        xt8_bc = xt8p.tile([128, kt2, 2, S_CHUNK], FP8, tag="xt8")
                        nc.sync.dma_start(xt8_bc[:], xt8_d[b, c])
                    xt8_tiles.append(xt8_bc)
                    # xtb rides the sync ring too: DMA issues on the scalar
                    # queue would steal ~1.8us/chunk from the tanh ACT chain
                    # that the ls matmuls gate on.
                    xtb_bc = xtbp.tile([128, kt, S_CHUNK], BF16, tag="xtb")
                    nc.sync.dma_start(xtb_bc[:], xtb_d[b, c])
                    xtb_tiles.append(xtb_bc)

                denom_b = smallp.tile([1, n_sc + 1], F32, tag="denom")
                ctx4_b = ctxp.tile([128, n_sc, kt], BF16, tag="ctx4")
                for c in range(n_sc):
                    ls_ps = ps_ls.tile([1, S_CHUNK], F32, tag="ls")
                    score_tiles = []
                    for j in range(jt):
                        mm_ps = ps_main.tile([128, S_CHUNK], F32, tag="main")
                        for t in range(kt2):
                            nc.tensor.matmul(
                                mm_ps[:],
                                lhsT=we_sb[:, j, t],
                                rhs=xt8_tiles[c][:, t],
                                start=(t == 0),
                                stop=(t == kt2 - 1),
                                perf_mode=DR,
                            )
                        if j % 2 == 0:
                            scp = scorep.tile([128, 2, S_CHUNK], FP8, tag="score")
                            score_tiles.append(scp)
                        nc.scalar.activation(
                            score_tiles[j // 2][:, j % 2], mm_ps[:], AF.Tanh,
                            bias=bias_sb[:, j * b_per_core + b : j * b_per_core + b + 1],
                            scale=1.0 / WE_SCALE,
                        )
                        if j == 2:
                            # deferred work from the previous chunk/batch is
                            # emitted two matmul groups in, so its ScalarE exp
                            # queues behind the tanh pair that the first ls
                            # matmul gates on, and the DVE context work
                            # overlaps this chunk's remaining matmul groups
                            for fn in pending:
                                fn()
                            pending = []
                        if j == 5 and pending_late:
                            # tail-context PE work flushes later still so its
                            # exp dependency has cleared the ScalarE queue
                            for fn in pending_late:
                                fn()
                            pending_late = []
                    for tj in range(jt2):
                        nc.tensor.matmul(
                            ls_ps[:],
                            lhsT=ws_sb[:, tj, :, 0:1],
                            rhs=score_tiles[tj][:],
                            start=(tj == 0),
                            stop=(tj == jt2 - 1),
                            perf_mode=DR,
                        )

                    if last_b and c == n_sc - 1:
                        # kernel tail: denominator + context via the idle PE
                        emit_pe_tail(ls_ps, denom_b, tailst)
                        emit_batch_final(b, ctx4_b, None, n_sc - 2)
                    elif last_b and c == n_sc - 2:
                        # the second-to-last chunk's context also runs on the
                        # PE (accumulating into the tail PSUM): its DVE reduce
                        # would otherwise spill ~5us past the matmul stream
                        def c2_exp(ls_ps=ls_ps, denom_b=denom_b, c=c):
                            tailst["ex2"] = emit_exp(ls_ps, denom_b, c)
                        pending.append(c2_exp)
                        def c2_pe():
                            ex2 = tailst["ex2"]
                            slices = [
                                ex2[:, si * 128 : (si + 1) * 128] for si in range(4)
                            ]
                            emit_pe_ctx(slices, xs_sb[:, 0], tailst, start=True)
                        pending_late.append(c2_pe)
                    elif c == n_sc - 1:
                        def batch_tail(ls_ps=ls_ps, b=b, c=c, ctx4_b=ctx4_b,
                                       denom_b=denom_b, xtb_bc=xtb_tiles[c]):
                            ex = emit_exp(ls_ps, denom_b, c)
                            invd_bc = emit_invd(denom_b, n_sc)
                            emit_context_chunk(xtb_bc, ex, ctx4_b, c)
                            emit_batch_final(b, ctx4_b, invd_bc, n_sc)
                        pending.append(batch_tail)
                    else:
                        def ctx_fn(ls_ps=ls_ps, c=c, ctx4_b=ctx4_b,
                                   denom_b=denom_b, xtb_bc=xtb_tiles[c]):
                            ex = emit_exp(ls_ps, denom_b, c)
                            emit_context_chunk(xtb_bc, ex, ctx4_b, c)
                        pending.append(ctx_fn)

    nc.compile()
    return nc


_CACHED = {}


def _get_program(key):
    if key not in _CACHED:
        _CACHED[key] = build_program(*key)
    return _CACHED[key]


def make_in_maps(encoder_out, decoder_hidden_state, We, be, Wd, bd, Ws, bs,
                 b_per_core=B_PER_CORE, s=S, h=H, n_cores=N_CORES):
    kt = h // 128
    kt2 = kt // 2
    jt = h // 128
    n_sc = s // S_CHUNK
    bf = ml_dtypes.bfloat16
    f8 = mybir.dt.np(FP8)

    # we8[j, p, t*256+r*128+m] = 128*We[(2t+r)*128+p, j*128+m]
    we8_a = np.ascontiguousarray(
        (We * WE_SCALE).reshape(kt2, 2, 128, jt, 128).transpose(3, 2, 0, 1, 4)
    ).reshape(jt, 128, h).astype(f8)
    # ws8[p, (t*2+r)*16] = 128*Ws[(2t+r)*128+p, 0]; 16-byte padded pair slabs
    ws8_a = np.zeros((128, kt2, 2, 16), dtype=f8)
    ws8_a[:, :, :, 0] = (
        (Ws[:, 0] * WE_SCALE).reshape(kt2, 2, 128).transpose(2, 0, 1).astype(f8)
    )
    ws8_a = ws8_a.reshape(128, kt2 * 2 * 16)

    dec = decoder_hidden_state[0]  # [32, h]
    bias_all = (be + bd)[None, :] + dec @ Wd  # [32, h] fp32
    in_maps = []
    for i in range(n_cores):
        b0 = i * b_per_core
        xb = encoder_out[b0 : b0 + b_per_core]  # [b, s, h]
        # fp8 PE copy: [b, c, s', t, r, p] -> [b, c, p, t, r, s']
        xt8_a = np.ascontiguousarray(
            xb.reshape(b_per_core, n_sc, S_CHUNK, kt2, 2, 128).transpose(0, 1, 5, 3, 4, 2)
        ).reshape(b_per_core, n_sc, 128, kt2 * 2 * S_CHUNK).astype(f8)
        # bf16 context copy: [b, c, s', k, p] -> [b, c, p, k, s']
        xtb_a = np.ascontiguousarray(
            xb.reshape(b_per_core, n_sc, S_CHUNK, kt, 128).transpose(0, 1, 4, 3, 2)
        ).reshape(b_per_core, n_sc, 128, kt * S_CHUNK).astype(bf)
        # s-major copy of the tail chunks (last batch, last TWO s-chunks),
        # p-major in DRAM so it ships as one contiguous DMA
        xs3_a = np.ascontiguousarray(
            xb[b_per_core - 1, (n_sc - 2) * S_CHUNK :]
            .reshape(2, S_CHUNK // 128, 128, h).transpose(2, 0, 1, 3)
        ).reshape(128, 2 * (S_CHUNK // 128) * h).astype(bf)
        # first chunk again, t-major contiguous slabs for the startup gate
        xt8f_a = np.ascontiguousarray(
            xt8_a[0, 0].reshape(128, kt2, 2 * S_CHUNK).transpose(1, 0, 2)
        )
        bias_a = np.ascontiguousarray(
            bias_all[b0 : b0 + b_per_core].reshape(b_per_core, jt, 128).transpose(2, 1, 0)
        ).reshape(128, jt * b_per_core).astype(np.float32)
        in_maps.append(
            {"xt8": xt8_a, "xtb": xtb_a, "we8": we8_a, "ws8": ws8_a,
             "bias": bias_a, "xs3": xs3_a, "xt8f": xt8f_a}
        )
    return in_maps


def kernel(encoder_out, decoder_hidden_state, We, be, Wd, bd, Ws, bs):
    encoder_out = np.asarray(encoder_out, dtype=np.float32)
    decoder_hidden_state = np.asarray(decoder_hidden_state, dtype=np.float32)
    We = np.asarray(We, dtype=np.float32)
    be = np.asarray(be, dtype=np.float32)
    Wd = np.asarray(Wd, dtype=np.float32)
    bd = np.asarray(bd, dtype=np.float32)
    Ws = np.asarray(Ws, dtype=np.float32)
    bs = np.asarray(bs, dtype=np.float32)

    nc = _get_program((B_PER_CORE, S, H))
    in_maps = make_in_maps(
        encoder_out, decoder_hidden_state, We, be, Wd, bd, Ws, bs
    )
    kwargs = {}
    if PROFILE["trace"]:
        kwargs = {"trace": True, "tmpdir": PROFILE["tmpdir"]}
    res = run_bass_kernel_spmd(nc, in_maps, list(range(N_CORES)), **kwargs)
    PROFILE["last_result"] = res

    out = np.empty((N_CORES * B_PER_CORE, H), dtype=np.float32)
    for i in range(N_CORES):
        r = res.results[i]
        ctx = r["ctx"]  # [b, 128, jt]
        out[i * B_PER_CORE : (i + 1) * B_PER_CORE] = (
            ctx.transpose(0, 2, 1).reshape(B_PER_CORE, H)
        )
        # batch 3: ctx[3] holds the unnormalized chunk 0-2 partial; add the
        # PE-tail chunk-3 partial and divide by the shipped denominator
        out[i * B_PER_CORE + B_PER_CORE - 1] = (
            ctx[B_PER_CORE - 1].T.reshape(H) + r["ctx3"][0, :H]
        ) / r["ctx3"][0, H]
    return out


# revision 19
# speedup vs baseline: 1.8730x; 1.0158x over previous
"""Bahdanau attention fused kernel for Trainium2, 8-core data-parallel.

Reference computation (per batch b of 32, H=1024, S=2048):
    enc_score = encoder_out @ We + be                    [B, S, H]
    dec_score = dec @ Wd + bd                            [B, 1, H]
    score     = tanh(enc_score + dec_score)              [B, S, H]
    ls        = score @ Ws + bs                          [B, S, 1]
    w         = softmax(ls, axis=S)
    out       = sum_s w[b,s] * encoder_out[b,s,:]        [B, H]

Sharding: batch 32 -> 4 per core across 8 cores; weights replicated.
The tiny dec-score GEMM is folded into the host-side bias preparation:
bias[b] = be + bd + dec[b] @ Wd. bs is dropped (softmax shift-invariant).

Numerics: the main GEMM and the ls projection run in fp8-e4m3 with
MatmulPerfMode.DoubleRow (two 128-deep k-tiles per matmul). We and Ws
are pre-scaled by 128 host-side to clear fp8 subnormals; the tanh/exp
activations apply scale=1/128 to compensate. X is shipped twice: fp8
(PE) and bf16 (VectorE context path). Total rel err ~1.7e-2 (sim-
verified; hardware matches the numpy fp8 sim to ~1e-5).

Per-core device layout (prepared host-side):
    xt8  [4, 4, 128, 4096] fp8  xt8[b,c,p,t*1024+r*512+s'] = X[b, c*512+s', (2t+r)*128+p]
    xtb  [4, 4, 128, 4096] bf16 xtb[b,c,p,k*512+s']        = X[b, c*512+s', k*128+p]
    we8  [8, 128, 1024]    fp8  we8[j,p,t*256+r*128+m]     = 128*We[(2t+r)*128+p, j*128+m]
    ws8  [128, 8]          fp8  ws8[p,t*2+r]               = 128*Ws[(2t+r)*128+p, 0]
    bias [128, 32]         f32  bias[p, j*4+b]             = (be+bd+dec[b]@Wd)[j*128+p]
    xs3  [4, 128, 1024]    bf16 xs3[si,p,n] = X[3, 3*512+si*128+p, n]   (tail chunk, s-major)
Outputs:
    ctx  [4, 128, 8] f32: batches 0-2 normalized contexts; batch 3 the
         UNNORMALIZED partial over chunks 0-2
    ctx3 [1, 1024]  f32: batch 3's unnormalized chunk-3 partial (h on free)
    den3 [1, 1]     f32: batch 3's softmax denominator
    (host: out[3] = (ctx[3].T + ctx3) / den3)

Device schedule per (batch, 512-wide s-chunk):
  - 8 j-groups x 4 DoubleRow matmuls accumulate enc_score.T in PSUM;
    ScalarE evacuates with fused tanh(psum/128 + bias) -> fp8 score
    pairs (j even/odd interleaved slabs for the ls DoubleRow rhs)
  - ls.T = 4 DoubleRow matmuls over score pairs -> PSUM [1,512]
  - the exp + context work for chunk c is DEFERRED into chunk c+1's
    matmul phase: the in-order ScalarE queue would otherwise stall on
    exp (which waits for the ls chain) ahead of the next chunk's tanh
    evacuations that the next ls matmuls gate on
  - context: exp weights broadcast to 128 partitions (GpSimd), one
    fused VectorE multiply [128,8,512] (broadcast-AP) + one fused
    per-k reduce -> ctx partials
  - kernel tail (last batch, last chunk): the context runs on the
    otherwise-idle PE instead of the DVE: exp halves -> PE transposes
    to [128,4] -> 8 matmuls against the s-major bf16 chunk -> [1,1024]
    unnormalized context, merged with the denominator on the host
"""

import numpy as np
import ml_dtypes

import concourse.tile as tile
from concourse import bacc, mybir
from concourse.bass_utils import run_bass_kernel_spmd

BF16 = mybir.dt.bfloat16
F32 = mybir.dt.float32
FP8 = mybir.dt.float8e4
AF = mybir.ActivationFunctionType
DR = mybir.MatmulPerfMode.DoubleRow

N_CORES = 8
H = 1024
S = 2048
B_PER_CORE = 4
S_CHUNK = 512
WE_SCALE = 128.0

# test.py can flip this to get a profiled run; the grading path never does.
PROFILE = {"trace": False, "tmpdir": None}


def build_program(b_per_core=B_PER_CORE, s=S, h=H):
    kt = h // 128          # 8  bf16 k-tiles (context path)
    kt2 = kt // 2          # 4  fp8 DoubleRow k-pair tiles
    jt = h // 128          # 8  output h tiles
    jt2 = jt // 2          # 4  score pair tiles (ls DoubleRow)
    n_sc = s // S_CHUNK    # 4  s chunks
    nc = bacc.Bacc("TRN2", target_bir_lowering=False, debug=False)

    xt8_d = nc.dram_tensor(
        "xt8", [b_per_core, n_sc, 128, kt2 * 2 * S_CHUNK], FP8, kind="ExternalInput"
    ).ap()
    # first chunk duplicated t-major: each k-pair slab is contiguous so the
    # first matmul group can start per-slab
    xt8f_d = nc.dram_tensor("xt8f", [kt2, 128, 2 * S_CHUNK], FP8, kind="ExternalInput").ap()
    xtb_d = nc.dram_tensor(
        "xtb", [b_per_core, n_sc, 128, kt * S_CHUNK], BF16, kind="ExternalInput"
    ).ap()
    # j-major so each per-j weight slab is one contiguous 128KB transfer
    we8_d = nc.dram_tensor("we8", [jt, 128, h], FP8, kind="ExternalInput").ap()
    # ws pair slabs padded to 16 bytes: DoubleRow ldweights requires the
    # k-pair step to be a multiple of 16 bytes
    ws8_d = nc.dram_tensor("ws8", [128, jt2 * 2 * 16], FP8, kind="ExternalInput").ap()
    bias_d = nc.dram_tensor(
        "bias", [128, jt * b_per_core], F32, kind="ExternalInput"
    ).ap()
    xs3_d = nc.dram_tensor("xs3", [128, 2 * (S_CHUNK // 128) * h], BF16, kind="ExternalInput").ap()
    ctx_d = nc.dram_tensor("ctx", [b_per_core, 128, jt], F32, kind="ExternalOutput").ap()
    # ctx3[0, :h] = unnormalized tail-chunk context; ctx3[0, h] = denominator
    ctx3_d = nc.dram_tensor("ctx3", [1, h + 1], F32, kind="ExternalOutput").ap()

    with tile.TileContext(nc) as tc:
        with (
            tc.tile_pool(name="consts", bufs=1) as consts,
            tc.tile_pool(name="xp", bufs=8) as xp,
            tc.tile_pool(name="scorep", bufs=8) as scorep,
            tc.tile_pool(name="smallp", bufs=2 * n_sc) as smallp,
            tc.tile_pool(name="vp", bufs=3) as vp,
            tc.tile_pool(name="ctxp", bufs=4) as ctxp,
            tc.tile_pool(name="ps_main", bufs=4, space="PSUM") as ps_main,
            tc.tile_pool(name="ps_ls", bufs=3, space="PSUM") as ps_ls,
            tc.tile_pool(name="ps_misc", bufs=1, space="PSUM") as ps_misc,
        ):
            # Gate-opening DMAs run on BOTH rings in parallel: the first
            # chunk (one contiguous 512KB) on sync, the weight slabs
            # (contiguous 128KB each, j-major) on scalar. The first matmul
            # gates on xt8[0,0] + we[j=0] only.
            xt8_first = xp.tile([128, kt2, 2, S_CHUNK], FP8, tag="xt8")
            for t in range(kt2):
                nc.sync.dma_start(xt8_first[:, t], xt8f_d[t])
            # ws/bias go FIRST on the scalar ring: the tanh chain gates on
            # bias, and a bias queued behind 1MB of weight slabs once stalled
            # the whole second chunk for 11us
            ws_sb = consts.tile([128, jt2, 2, 16], FP8)
            nc.scalar.dma_start(ws_sb[:], ws8_d[:])
            bias_sb = consts.tile([128, jt * b_per_core], F32)
            nc.scalar.dma_start(bias_sb[:], bias_d[:])
            we_sb = consts.tile([128, jt, kt2, 2, 128], FP8)
            for j in range(0, jt, 2):
                nc.scalar.dma_start(we_sb[:, j], we8_d[j])
            for j in range(1, jt, 2):
                nc.sync.dma_start(we_sb[:, j], we8_d[j])
            xs_sb = consts.tile([128, 2, S_CHUNK // 128, h], BF16)
            ones_bf = consts.tile([1, 128], BF16)
            nc.vector.memset(ones_bf[:], 1.0)
            ones_f32 = consts.tile([1, 128], F32)
            nc.vector.memset(ones_f32[:], 1.0)
            # PE warmup during the startup DMA gate: full-array dummy
            # matmuls keep the HAM activity monitor busy so the real stream
            # starts at the full 2.4GHz instead of paying ~20 cold matmuls
            # at 1.2GHz (K=1 dummies don't move the activity counter)
            dum_w = consts.tile([128, 128], BF16)
            nc.vector.memset(dum_w[:], 0.0)
            dum_x = consts.tile([128, S_CHUNK], BF16)
            nc.vector.memset(dum_x[:], 0.0)
            warm_ps = ps_misc.tile([128, S_CHUNK], F32, tag="misc")
            for _ in range(12):
                nc.tensor.matmul(
                    warm_ps[:], lhsT=dum_w[:], rhs=dum_x[:], start=True, stop=True
                )

            def emit_exp(ls_ps, denom_b, c):
                """exp(ls/128) -> bf16 weights + f32 denominator slot."""
                ex = smallp.tile([1, S_CHUNK], BF16, tag="exp")
                nc.scalar.activation(
                    ex[:], ls_ps[:], AF.Exp, scale=1.0 / WE_SCALE,
                    accum_out=denom_b[:, c : c + 1],
                )
                return ex

            def emit_context_chunk(xtb_bc, ex, ctx4_b, c):
                """Broadcast chunk weights (GpSimd), then one fused multiply
                + one fused per-k reduce for the whole chunk (DVE)."""
                ebc = vp.tile([128, S_CHUNK], BF16, tag="ebc")
                nc.gpsimd.partition_broadcast(ebc[:], ex[:])
                scr = vp.tile([128, kt, S_CHUNK], BF16, tag="scr")
                ebc_b = ebc[:].unsqueeze(1).broadcast_to((128, kt, S_CHUNK))
                nc.vector.tensor_mul(scr[:], xtb_bc[:], ebc_b)
                # bf16 partials keep the reduce in the DVE's 2x perf mode
                # (2-byte dst required); costs ~0.2% relative error
                with nc.allow_low_precision("bf16 context partials"):
                    nc.vector.reduce_sum(
                        ctx4_b[:, c], scr[:], axis=mybir.AxisListType.X
                    )

            def emit_invd(denom_b, width):
                """softmax denominator -> broadcast 1/d [128, 1]."""
                dsum = smallp.tile([1, 1], F32, tag="dsum")
                nc.vector.reduce_sum(
                    dsum[:], denom_b[:, :width], axis=mybir.AxisListType.X
                )
                invd = smallp.tile([1, 1], F32, tag="invd")
                nc.vector.reciprocal(invd[:], dsum[:])
                iv_ps = ps_misc.tile([128, S_CHUNK], F32, tag="misc")
                nc.tensor.matmul(
                    iv_ps[:, 0:1], lhsT=ones_f32[:], rhs=invd[:], start=True, stop=True
                )
                invd_bc = smallp.tile([128, 1], F32, tag="invdbc")
                nc.scalar.copy(invd_bc[:], iv_ps[:, 0:1])
                return invd_bc

            def emit_batch_final(b, ctx4_b, invd_bc, width):
                """Partial reduction over chunks, normalize, store."""
                ctxu = ctxp.tile([128, jt], F32, tag="ctxu")
                nc.vector.reduce_sum(
                    ctxu[:],
                    ctx4_b[:, :width].transpose([0, 2, 1]),
                    axis=mybir.AxisListType.X,
                )
                if invd_bc is None:
                    nc.sync.dma_start(ctx_d[b], ctxu[:])
                else:
                    ctx_b = ctxp.tile([128, jt], F32, tag="ctx")
                    nc.vector.tensor_scalar_mul(ctx_b[:], ctxu[:], invd_bc[:])
                    nc.sync.dma_start(ctx_d[b], ctx_b[:])

            def emit_pe_ctx(ex_slices, xs_half, tailst, start):
                """One chunk of tail context on the PE: 4 transposes of the
                exp weights into exT [128,4], then 8 matmuls against the
                s-major bf16 chunk, accumulating [1,1024] across chunks in
                shared PSUM (start on the first chunk, stop on the last)."""
                exT_ps = ps_misc.tile([128, 4, 2], BF16, tag="misc")
                for si in range(4):
                    nc.tensor.transpose(
                        exT_ps[:, si, 0:1], ex_slices[si], ones_bf[:, 0:1]
                    )
                exT = smallp.tile([128, 4], BF16, tag="exT_sb")
                nc.scalar.copy(exT[:], exT_ps[:, :, 0])
                if start:
                    tailst["ctx_ps"] = [
                        ps_ls.tile([1, S_CHUNK], F32, tag="ls", name=f"ctx3ps{hh}")
                        for hh in range(2)
                    ]
                for hh in range(2):
                    c3_ps = tailst["ctx_ps"][hh]
                    for si in range(4):
                        nc.tensor.matmul(
                            c3_ps[:],
                            lhsT=exT[:, si : si + 1],
                            rhs=xs_half[:, si, hh * S_CHUNK : (hh + 1) * S_CHUNK],
                            start=(start and si == 0),
                            stop=((not start) and si == 3),
                            skip_group_check=True,
                        )

            def emit_pe_tail(ls_ps, denom_b, tailst):
                """Kernel tail: the last chunk's exp in two halves, its
                context on the idle PE, then the combined chunk-2+3 partial
                and the denominator ship for the host-side divide."""
                ex_h = []
                for hh in range(2):
                    exh = smallp.tile([1, 256], BF16, tag="exp")
                    nc.scalar.activation(
                        exh[:], ls_ps[:, hh * 256 : (hh + 1) * 256], AF.Exp,
                        scale=1.0 / WE_SCALE,
                        accum_out=denom_b[:, n_sc - 1 + hh : n_sc + hh],
                    )
                    ex_h.append(exh)
                slices = [
                    ex_h[si // 2][:, (si % 2) * 128 : (si % 2 + 1) * 128]
                    for si in range(4)
                ]
                emit_pe_ctx(slices, xs_sb[:, 1], tailst, start=False)
                ctx3_sb = ctxp.tile([1, h + 1], F32, tag="ctx3")
                nc.scalar.copy(ctx3_sb[:, :S_CHUNK], tailst["ctx_ps"][0][:])
                nc.vector.tensor_copy(
                    ctx3_sb[:, S_CHUNK : 2 * S_CHUNK], tailst["ctx_ps"][1][:]
                )
                # denominator: chunks 0-2 in slots 0-2, chunk 3 halves in 3-4
                nc.vector.reduce_sum(
                    ctx3_sb[:, h : h + 1], denom_b[:], axis=mybir.AxisListType.X
                )
                nc.sync.dma_start(ctx3_d[:], ctx3_sb[:])

            pending = []  # deferred (exp | context-chunk | invd | batch-final)
            pending_late = []  # deferred PE tail-context work (flushed at j==5)
            tailst = {}
            for b in range(b_per_core):
                last_b = b == b_per_core - 1
                if last_b:
                    nc.sync.dma_start(xs_sb[:], xs3_d[:])
                xt8_tiles = []
                xtb_tiles = []
                for c in range(n_sc):
                    if b == 0 and c == 0:
                        xt8_bc = xt8_first
                    else:
                        xt8_bc = xp.tile([128, kt2, 2, S_CHUNK], FP8, tag="xt8")
                        nc.sync.dma_start(xt8_bc[:], xt8_d[b, c])
                    xt8_tiles.append(xt8_bc)
                    # xtb rides the sync ring too: DMA issues on the scalar
                    # queue would steal ~1.8us/chunk from the tanh ACT chain
                    # that the ls matmuls gate on.
                    xtb_bc = xp.tile([128, kt, S_CHUNK], BF16, tag="xtb")
                    nc.sync.dma_start(xtb_bc[:], xtb_d[b, c])
                    xtb_tiles.append(xtb_bc)

                denom_b = smallp.tile([1, n_sc + 1], F32, tag="denom")
                ctx4_b = ctxp.tile([128, n_sc, kt], BF16, tag="ctx4")
                for c in range(n_sc):
                    ls_ps = ps_ls.tile([1, S_CHUNK], F32, tag="ls")
                    score_tiles = []
                    for j in range(jt):
                        mm_ps = ps_main.tile([128, S_CHUNK], F32, tag="main")
                        for t in range(kt2):
                            nc.tensor.matmul(
                                mm_ps[:],
                                lhsT=we_sb[:, j, t],
                                rhs=xt8_tiles[c][:, t],
                                start=(t == 0),
                                stop=(t == kt2 - 1),
                                perf_mode=DR,
                            )
                        if j % 2 == 0:
                            scp = scorep.tile([128, 2, S_CHUNK], FP8, tag="score")
                            score_tiles.append(scp)
                        nc.scalar.activation(
                            score_tiles[j // 2][:, j % 2], mm_ps[:], AF.Tanh,
                            bias=bias_sb[:, j * b_per_core + b : j * b_per_core + b + 1],
                            scale=1.0 / WE_SCALE,
                        )
                        if j == 2:
                            # deferred work from the previous chunk/batch is
                            # emitted two matmul groups in, so its ScalarE exp
                            # queues behind the tanh pair that the first ls
                            # matmul gates on, and the DVE context work
                            # overlaps this chunk's remaining matmul groups
                            for fn in pending:
                                fn()
                            pending = []
                        if j == 5 and pending_late:
                            # tail-context PE work flushes later still so its
                            # exp dependency has cleared the ScalarE queue
                            for fn in pending_late:
                                fn()
                            pending_late = []
                    for tj in range(jt2):
                        nc.tensor.matmul(
                            ls_ps[:],
                            lhsT=ws_sb[:, tj, :, 0:1],
                            rhs=score_tiles[tj][:],
                            start=(tj == 0),
                            stop=(tj == jt2 - 1),
                            perf_mode=DR,
                        )

                    if last_b and c == n_sc - 1:
                        # kernel tail: denominator + context via the idle PE
                        emit_pe_tail(ls_ps, denom_b, tailst)
                        emit_batch_final(b, ctx4_b, None, n_sc - 2)
                    elif last_b and c == n_sc - 2:
                        # the second-to-last chunk's context also runs on the
                        # PE (accumulating into the tail PSUM): its DVE reduce
                        # would otherwise spill ~5us past the matmul stream
                        def c2_exp(ls_ps=ls_ps, denom_b=denom_b, c=c):
                            tailst["ex2"] = emit_exp(ls_ps, denom_b, c)
                        pending.append(c2_exp)
                        def c2_pe():
                            ex2 = tailst["ex2"]
                            slices = [
                                ex2[:, si * 128 : (si + 1) * 128] for si in range(4)
                            ]
                            emit_pe_ctx(slices, xs_sb[:, 0], tailst, start=True)
                        pending_late.append(c2_pe)
                    elif c == n_sc - 1:
                        def batch_tail(ls_ps=ls_ps, b=b, c=c, ctx4_b=ctx4_b,
                                       denom_b=denom_b, xtb_bc=xtb_tiles[c]):
                            ex = emit_exp(ls_ps, denom_b, c)
                            invd_bc = emit_invd(denom_b, n_sc)
                            emit_context_chunk(xtb_bc, ex, ctx4_b, c)
                            emit_batch_final(b, ctx4_b, invd_bc, n_sc)
                        pending.append(batch_tail)
                    else:
                        def ctx_fn(ls_ps=ls_ps, c=c, ctx4_b=ctx4_b,
                                   denom_b=denom_b, xtb_bc=xtb_tiles[c]):
                            ex = emit_exp(ls_ps, denom_b, c)
                            emit_context_chunk(xtb_bc, ex, ctx4_b, c)
                        pending.append(ctx_fn)

    nc.compile()
    return nc


_CACHED = {}


def _get_program(key):
    if key not in _CACHED:
        _CACHED[key] = build_program(*key)
    return _CACHED[key]


def make_in_maps(encoder_out, decoder_hidden_state, We, be, Wd, bd, Ws, bs,
                 b_per_core=B_PER_CORE, s=S, h=H, n_cores=N_CORES):
    kt = h // 128
    kt2 = kt // 2
    jt = h // 128
    n_sc = s // S_CHUNK
    bf = ml_dtypes.bfloat16
    f8 = mybir.dt.np(FP8)

    # we8[j, p, t*256+r*128+m] = 128*We[(2t+r)*128+p, j*128+m]
    we8_a = np.ascontiguousarray(
        (We * WE_SCALE).reshape(kt2, 2, 128, jt, 128).transpose(3, 2, 0, 1, 4)
    ).reshape(jt, 128, h).astype(f8)
    # ws8[p, (t*2+r)*16] = 128*Ws[(2t+r)*128+p, 0]; 16-byte padded pair slabs
    ws8_a = np.zeros((128, kt2, 2, 16), dtype=f8)
    ws8_a[:, :, :, 0] = (
        (Ws[:, 0] * WE_SCALE).reshape(kt2, 2, 128).transpose(2, 0, 1).astype(f8)
    )
    ws8_a = ws8_a.reshape(128, kt2 * 2 * 16)

    dec = decoder_hidden_state[0]  # [32, h]
    bias_all = (be + bd)[None, :] + dec @ Wd  # [32, h] fp32
    in_maps = []
    for i in range(n_cores):
        b0 = i * b_per_core
        xb = encoder_out[b0 : b0 + b_per_core]  # [b, s, h]
        # fp8 PE copy: [b, c, s', t, r, p] -> [b, c, p, t, r, s']
        xt8_a = np.ascontiguousarray(
            xb.reshape(b_per_core, n_sc, S_CHUNK, kt2, 2, 128).transpose(0, 1, 5, 3, 4, 2)
        ).reshape(b_per_core, n_sc, 128, kt2 * 2 * S_CHUNK).astype(f8)
        # bf16 context copy: [b, c, s', k, p] -> [b, c, p, k, s']
        xtb_a = np.ascontiguousarray(
            xb.reshape(b_per_core, n_sc, S_CHUNK, kt, 128).transpose(0, 1, 4, 3, 2)
        ).reshape(b_per_core, n_sc, 128, kt * S_CHUNK).astype(bf)
        # s-major copy of the tail chunks (last batch, last TWO s-chunks),
        # p-major in DRAM so it ships as one contiguous DMA
        xs3_a = np.ascontiguousarray(
            xb[b_per_core - 1, (n_sc - 2) * S_CHUNK :]
            .reshape(2, S_CHUNK // 128, 128, h).transpose(2, 0, 1, 3)
        ).reshape(128, 2 * (S_CHUNK // 128) * h).astype(bf)
        # first chunk again, t-major contiguous slabs for the startup gate
        xt8f_a = np.ascontiguousarray(
            xt8_a[0, 0].reshape(128, kt2, 2 * S_CHUNK).transpose(1, 0, 2)
        )
        bias_a = np.ascontiguousarray(
            bias_all[b0 : b0 + b_per_core].reshape(b_per_core, jt, 128).transpose(2, 1, 0)
        ).reshape(128, jt * b_per_core).astype(np.float32)
        in_maps.append(
            {"xt8": xt8_a, "xtb": xtb_a, "we8": we8_a, "ws8": ws8_a,
             "bias": bias_a, "xs3": xs3_a, "xt8f": xt8f_a}
        )
    return in_maps


def kernel(encoder_out, decoder_hidden_state, We, be, Wd, bd, Ws, bs):
    encoder_out = np.asarray(encoder_out, dtype=np.float32)
    decoder_hidden_state = np.asarray(decoder_hidden_state, dtype=np.float32)
    We = np.asarray(We, dtype=np.float32)
    be = np.asarray(be, dtype=np.float32)
    Wd = np.asarray(Wd, dtype=np.float32)
    bd = np.asarray(bd, dtype=np.float32)
    Ws = np.asarray(Ws, dtype=np.float32)
    bs = np.asarray(bs, dtype=np.float32)

    nc = _get_program((B_PER_CORE, S, H))
    in_maps = make_in_maps(
        encoder_out, decoder_hidden_state, We, be, Wd, bd, Ws, bs
    )
    kwargs = {}
    if PROFILE["trace"]:
        kwargs = {"trace": True, "tmpdir": PROFILE["tmpdir"]}
    res = run_bass_kernel_spmd(nc, in_maps, list(range(N_CORES)), **kwargs)
    PROFILE["last_result"] = res

    out = np.empty((N_CORES * B_PER_CORE, H), dtype=np.float32)
    for i in range(N_CORES):
        r = res.results[i]
        ctx = r["ctx"]  # [b, 128, jt]
        out[i * B_PER_CORE : (i + 1) * B_PER_CORE] = (
            ctx.transpose(0, 2, 1).reshape(B_PER_CORE, H)
        )
        # batch 3: ctx[3] holds the unnormalized chunk 0-2 partial; add the
        # PE-tail chunk-3 partial and divide by the shipped denominator
        out[i * B_PER_CORE + B_PER_CORE - 1] = (
            ctx[B_PER_CORE - 1].T.reshape(H) + r["ctx3"][0, :H]
        ) / r["ctx3"][0, H]
    return out
